# revision 2
# baseline (speedup 1.0000x reference)
"""MLA multi-head latent attention kernel for 8 TRN2 NeuronCores, v2.

Sharding: 8 cores = 2 batches (DP) x 4 head-groups of 4 heads (TP).
v2 removes the 4x-replicated shared LoRA down-projection: each core computes
x @ [q_a | kv_a | k_rope] only for ITS T-quarter (pre-normalized, RMS rsqrt
applied before staging), then two HBM AllGathers across the 4-core TP group
(kv scratch first so kv_b starts while the larger q gather is in flight)
reassemble the full-T scratch. Everything else (q_b, kv_b, attention for the
core's 4 heads, partial o_proj) is head-sharded as before; host sums the 4
per-group partial outputs per batch.

Other changes vs the 702us baseline: q_b outputs stay SBUF-resident (no DRAM
round-trip), softmax denominators accumulate on DVE + one ones-matmul
(instead of a per-k-tile PE matmul), reciprocal runs on the Act engine (DVE
RECIPROCAL is ~4us/tile), o_proj is interleaved per query-tile with
attention, and the partial outputs are written bf16.
"""
import math
import sys
from contextlib import ExitStack
from dataclasses import dataclass

sys.path.insert(0, '/opt/trn_rl_repo')
import numpy as np
import ml_dtypes
import concourse.bass as bass
import concourse.bacc as bacc
import concourse.mybir as mybir
from concourse import tile
from concourse.bass_utils import run_bass_kernel_spmd

F32 = mybir.dt.float32
BF16 = mybir.dt.bfloat16
FP8 = mybir.dt.float8e4
AF = mybir.ActivationFunctionType


@dataclass
class Cfg:
    T: int = 2048
    D: int = 2048
    QL: int = 1536
    KVL: int = 512
    NHC: int = 4          # heads per core
    NOPE: int = 128
    ROPE: int = 64
    V: int = 128
    eps: float = 1e-6
    rope_base: float = 10000.0

    @property
    def NC(self):
        return min(512, self.T)

    @property
    def KD(self):
        return self.D // 128

    @property
    def MQ(self):
        return self.QL // 128

    @property
    def MKV(self):
        return self.KVL // 128

    @property
    def NT(self):
        return self.T // self.NC

    @property
    def TK(self):
        return self.T // 128

    @property
    def TQ(self):         # local T-quarter (phase-1 shard)
        return self.T // 4

    @property
    def MASKW(self):
        return 2 * self.NC - 128

    @property
    def QH(self):
        return self.NOPE + self.ROPE

    @property
    def NQM(self):        # q_b output 128-chunks
        return (self.NHC * self.QH) // 128

    @property
    def KVROWS(self):     # rows in the kv gather: kv_c + krope
        return self.KVL + self.ROPE


# full-scale problem constants (per harness contract)
B, T, D = 2, 2048, 2048
QL, KVL = 1536, 512
NHEADS, NOPE, ROPE, V = 16, 128, 64, 128
QH = NOPE + ROPE
NCORES = 8
GROUPS = 4
NHC = NHEADS // GROUPS
FULL = Cfg()
REPLICA_GROUPS = [[0, 1, 2, 3], [4, 5, 6, 7]]


def build_nc(c: Cfg = FULL, num_devices: int = NCORES):
    nc = bacc.Bacc("TRN2", target_bir_lowering=False, debug=False,
                   num_devices=num_devices)
    W1C = c.QL + c.KVL + c.ROPE

    NCH = (W1C + 127) // 128           # w1 output chunks (last is 64 wide)
    xT = nc.dram_tensor("xT", [c.D, c.TQ], BF16, kind="ExternalInput").ap()
    # w1 pre-tiled on host: [p, chunk, k, col] so each chunk load is one
    # contiguous 4KB-per-partition descriptor instead of 256B strided rows
    w1 = nc.dram_tensor("w1", [128, NCH, c.KD, 128], BF16,
                        kind="ExternalInput").ap()
    qbw = nc.dram_tensor("qbw", [c.QL, c.NHC * c.QH], BF16, kind="ExternalInput").ap()
    kbw = nc.dram_tensor("kbw", [c.KVL, c.NHC * 128], BF16, kind="ExternalInput").ap()
    vbw = nc.dram_tensor("vbw", [c.KVL, c.NHC * c.V], BF16, kind="ExternalInput").ap()
    ow = nc.dram_tensor("ow", [c.NHC * c.V, c.D], BF16, kind="ExternalInput").ap()
    cos2 = nc.dram_tensor("cos2", [128, c.T], BF16, kind="ExternalInput").ap()
    sin2 = nc.dram_tensor("sin2", [128, c.T], BF16, kind="ExternalInput").ap()
    perm = nc.dram_tensor("perm", [128, 128], BF16, kind="ExternalInput").ap()
    maskt = nc.dram_tensor("maskt", [128, c.MASKW], F32, kind="ExternalInput").ap()
    outT = nc.dram_tensor("outT", [c.D, c.T], BF16, kind="ExternalOutput").ap()

    m1 = []
    off = 0
    while off < W1C:
        sz = min(128, W1C - off)
        m1.append((off, sz))
        off += sz
    NKCH = len(m1) - c.MQ              # kv chunk count (incl krope partial)
    m_order = list(range(c.MQ, len(m1))) + list(range(c.MQ))  # kv chunks first

    with tile.TileContext(nc) as tc, ExitStack() as top:
        dram = top.enter_context(tc.tile_pool(name="dram", bufs=1, space="DRAM"))
        g1_in = dram.tile([c.KVROWS, c.TQ], BF16)
        g2_in = dram.tile([c.QL, c.TQ], FP8)
        g1_out = dram.tile([4 * c.KVROWS, c.TQ], BF16)
        g2_out = dram.tile([4 * c.QL, c.TQ], FP8)

        const = top.enter_context(tc.tile_pool(name="const", bufs=1))
        ones_f = const.tile([128, 128], F32)
        nc.vector.memset(ones_f[:], 1.0)
        ones = const.tile([128, 128], BF16)
        nc.vector.tensor_copy(ones[:], ones_f[:])
        eps_sb = const.tile([128, 1], F32)
        nc.vector.memset(eps_sb[:], float(c.eps))

        # persistent SBUF: per-head K/V, q_b outputs, rope tables, weights
        kvc = top.enter_context(tc.tile_pool(name="kvc", bufs=1))
        knope = [kvc.tile([128, c.T], BF16, tag=f"kn{i}", name=f"kn{i}")
                 for i in range(c.NHC)]
        krope = kvc.tile([128, c.T], BF16, tag="krope")  # duplicated halves
        vsb = [kvc.tile([128, c.NHC * c.V], BF16, tag=f"v{i}", name=f"v{i}")
               for i in range(c.TK)]
        q_sb = [kvc.tile([128, c.T], BF16, tag=f"qs{m}", name=f"qs{m}")
                for m in range(c.NQM)]

        tb_pool = top.enter_context(tc.tile_pool(name="ropetb", bufs=1))
        cos_sb = tb_pool.tile([128, c.T], BF16, tag="cos")
        sin_sb = tb_pool.tile([128, c.T], BF16, tag="sin")
        perm_sb = tb_pool.tile([128, 128], BF16, tag="perm")

        wpool = top.enter_context(tc.tile_pool(name="wts", bufs=1))
        kbw_sb = [wpool.tile([128, c.NHC * 128], BF16, tag=f"kbw{k}",
                             name=f"kbw{k}") for k in range(c.MKV)]
        vbw_sb = [wpool.tile([128, c.NHC * c.V], BF16, tag=f"vbw{k}",
                             name=f"vbw{k}") for k in range(c.MKV)]
        qbw_sb = [wpool.tile([128, c.NHC * c.QH], BF16, tag=f"qbw{k}",
                             name=f"qbw{k}") for k in range(c.MQ)]
        ow_sb = [wpool.tile([128, c.D], BF16, tag=f"ow{h}", name=f"ow{h}")
                 for h in range(c.NHC)]
        mask_sb = wpool.tile([128, c.MASKW], F32, tag="mask")

        # weight/table loads, interleaved 1-2 per phase-1 chunk iteration on
        # the sync queues (need-order: tables/mask, kv_b, q_b, o_proj)
        wload = ([(cos_sb, cos2), (sin_sb, sin2), (perm_sb, perm),
                  (mask_sb, maskt)]
                 + [(kbw_sb[k], kbw[k * 128:(k + 1) * 128, :])
                    for k in range(c.MKV)]
                 + [(vbw_sb[k], vbw[k * 128:(k + 1) * 128, :])
                    for k in range(c.MKV)]
                 + [(qbw_sb[k], qbw[k * 128:(k + 1) * 128, :])
                    for k in range(c.MQ)]
                 + [(ow_sb[h], ow[h * c.V:(h + 1) * c.V, :])
                    for h in range(c.NHC)])

        # ---------------- phase 1: local-quarter x @ [q_a | kv_a | k_rope] --
        with ExitStack() as p1:
            xt_pool = p1.enter_context(tc.tile_pool(name="xt", bufs=1))
            w1_pool = p1.enter_context(tc.tile_pool(name="w1", bufs=3))
            ev_pool = p1.enter_context(tc.tile_pool(name="p1ev", bufs=1))
            sq_pool = p1.enter_context(tc.tile_pool(name="p1sq", bufs=3))
            nrm_pool = p1.enter_context(tc.tile_pool(name="p1nrm", bufs=3))
            rsq_pool = p1.enter_context(tc.tile_pool(name="p1rsq", bufs=1))
            ps_pool = p1.enter_context(tc.tile_pool(name="p1ps", bufs=2, space="PSUM"))
            ssq_ps = p1.enter_context(tc.tile_pool(name="ssqps", bufs=2, space="PSUM"))

            xt_sb = [xt_pool.tile([128, c.TQ], BF16, tag=f"xt{k}", name=f"xt{k}")
                     for k in range(c.KD)]
            for k in range(4):   # rest issued inside the first chunk iteration
                nc.sync.dma_start(xt_sb[k][:], xT[k * 128:(k + 1) * 128, :])

            ev_sb = [ev_pool.tile([128, c.TQ], BF16, tag=f"ev{i}", name=f"ev{i}")
                     for i in range(len(m1))]
            rsq_q = rsq_pool.tile([128, c.TQ], F32, tag="rsq_q")
            rsq_kv = rsq_pool.tile([128, c.TQ], F32, tag="rsq_kv")
            ssq_q = ssq_ps.tile([128, c.TQ], F32, tag="ssq_q", name="ssq_q")
            ssq_kv = ssq_ps.tile([128, c.TQ], F32, tag="ssq_kv", name="ssq_kv")

            def flush_group(is_q):
                # rsq from accumulated ssq, normalize chunk tiles, stage to
                # the gather input, then launch the group's AllGather
                dim = c.QL if is_q else c.KVL
                tgt = rsq_q if is_q else rsq_kv
                ssq = ssq_q if is_q else ssq_kv
                nc.scalar.activation(tgt[:], ssq[:], AF.Sqrt,
                                     bias=eps_sb[:], scale=1.0 / dim)
                nc.vector.reciprocal_approx_fast(tgt[:], tgt[:])
                idxs = range(c.MQ) if is_q else range(c.MQ, c.MQ + c.MKV)
                gdst = g2_in if is_q else g1_in
                for j, mi in enumerate(idxs):
                    # q scratch is gathered in fp8 (halves the collective's
                    # serial transfer time; ~1.6e-2 end-to-end error, within
                    # the 2e-2 budget); kv scratch stays bf16
                    nt = nrm_pool.tile([128, c.TQ], FP8 if is_q else BF16,
                                       tag="ntq" if is_q else "nt")
                    nc.vector.tensor_mul(nt[:], ev_sb[mi][:], tgt[:])
                    nc.sync.dma_start(gdst[j * 128:(j + 1) * 128, :], nt[:])
                if not is_q:
                    # krope rows staged raw (not RMS-normalized)
                    kr = ev_sb[c.MQ + c.MKV]
                    nc.sync.dma_start(g1_in[c.KVL:c.KVL + c.ROPE, :],
                                      kr[0:c.ROPE, :])
                gin, gout = (g2_in, g2_out) if is_q else (g1_in, g1_out)
                # flatten to 1-D so the CC engine sees one contiguous
                # buffer (large bursts) instead of 1KB rows
                nc.gpsimd.collective_compute(
                    "AllGather", mybir.AluOpType.bypass,
                    replica_groups=REPLICA_GROUPS,
                    ins=[gin[:].rearrange("a b -> (a b)")],
                    outs=[gout[:].rearrange("a b -> (a b)")])

            NW = len(wload)
            for oi, mi in enumerate(m_order):
                m0, msz = m1[mi]
                wt = w1_pool.tile([128, c.KD, 128], BF16, tag="w1t")
                nc.sync.dma_start(wt[:], w1[:, mi])
                if oi == 0:
                    for k in range(4, c.KD):
                        nc.sync.dma_start(xt_sb[k][:],
                                          xT[k * 128:(k + 1) * 128, :])
                lo = NW * oi // len(m_order)
                hi = NW * (oi + 1) // len(m_order)
                for dst, src in wload[lo:hi]:
                    nc.sync.dma_start(dst[:], src)
                ps = ps_pool.tile([128, c.TQ], F32, tag="ps")
                for k in range(c.KD):
                    nc.tensor.matmul(ps[:msz, :], wt[:, k, :msz],
                                     xt_sb[k][:],
                                     start=(k == 0), stop=(k == c.KD - 1))
                nc.scalar.copy(ev_sb[mi][:msz, :], ps[:msz, :])
                is_q = mi < c.MQ
                is_kr = m0 >= c.QL + c.KVL
                if not is_kr:
                    sq = sq_pool.tile([128, c.TQ], BF16, tag="sq")
                    nc.scalar.square(sq[:msz, :], ps[:msz, :])
                    ssq = ssq_q if is_q else ssq_kv
                    nmax = c.MQ if is_q else c.MKV
                    mloc = mi if is_q else mi - c.MQ
                    nc.tensor.matmul(ssq[:], ones[:msz, :], sq[:msz, :],
                                     start=(mloc == 0), stop=(mloc == nmax - 1))
                if oi == NKCH - 1:      # all kv chunks (incl krope) evicted
                    flush_group(is_q=False)
            flush_group(is_q=True)

        # ---------------- phases 2a/2b/3/4 ----------------
        rt_pool = top.enter_context(tc.tile_pool(name="p2rt", bufs=4))

        def rope_cols(x_ap, ns, pspool):
            # in-place rotate-half on [128, NC] slice: the partition swap is
            # a PE permutation matmul (no SBUF-SBUF DMA round-trips); sign
            # is folded into the sin table
            pr = pspool.tile([128, c.NC], F32, tag="ps")
            nc.tensor.matmul(pr[:], perm_sb[:], x_ap, start=True, stop=True)
            tmp = rt_pool.tile([128, c.NC], BF16, tag="rtmp", name="rtmp")
            t1 = rt_pool.tile([128, c.NC], BF16, tag="rt1", name="rt1")
            nc.vector.tensor_mul(tmp[:], pr[:], sin_sb[:, ns])
            nc.vector.tensor_mul(t1[:], x_ap, cos_sb[:, ns])
            nc.vector.tensor_add(x_ap, t1[:], tmp[:])

        # ---- 2a: kv_b (reads the kv gather only) ----
        with ExitStack() as p2a:
            kvr_pool = p2a.enter_context(tc.tile_pool(name="kvr", bufs=1))
            ps2 = p2a.enter_context(tc.tile_pool(name="ps2", bufs=2, space="PSUM"))
            kv_raw = [kvr_pool.tile([128, c.T], BF16, tag=f"kvr{k}",
                                    name=f"kvr{k}") for k in range(c.MKV)]
            for qtr in range(4):        # quarter-major so knope starts early
                for k in range(c.MKV):
                    nc.sync.dma_start(
                        kv_raw[k][:, qtr * c.TQ:(qtr + 1) * c.TQ],
                        g1_out[qtr * c.KVROWS + k * 128:
                               qtr * c.KVROWS + (k + 1) * 128, :])
            for qtr in range(4):
                src = g1_out[qtr * c.KVROWS + c.KVL:
                             qtr * c.KVROWS + c.KVL + c.ROPE, :]
                nc.sync.dma_start(krope[0:64, qtr * c.TQ:(qtr + 1) * c.TQ], src)
                nc.sync.dma_start(krope[64:128, qtr * c.TQ:(qtr + 1) * c.TQ], src)
            for n in range(c.NT):
                ns = slice(n * c.NC, (n + 1) * c.NC)
                rope_cols(krope[:, ns], ns, ps2)

            for h in range(c.NHC):
                for n in range(c.NT):
                    ns = slice(n * c.NC, (n + 1) * c.NC)
                    ps = ps2.tile([128, c.NC], F32, tag="ps")
                    for k in range(c.MKV):
                        nc.tensor.matmul(ps[:], kbw_sb[k][:, h * 128:(h + 1) * 128],
                                         kv_raw[k][:, ns],
                                         start=(k == 0), stop=(k == c.MKV - 1))
                    nc.scalar.copy(knope[h][:, ns], ps[:])
            for m in range(c.TK):
                ms = slice(m * 128, (m + 1) * 128)
                ps = ps2.tile([128, c.NHC * c.V], F32, tag="ps")
                for k in range(c.MKV):
                    nc.tensor.matmul(ps[:], kv_raw[k][:, ms], vbw_sb[k][:],
                                     start=(k == 0), stop=(k == c.MKV - 1))
                nc.scalar.copy(vsb[m][:], ps[:])

        # ---- interleaved q_b(quarter) -> attention(qn) -> o_proj(pair) ----
        # the q AllGather delivers peer quarters progressively at CC speed;
        # consuming quarter n right before attention qn hides the CC tail
        # behind attention compute instead of stalling after phase 2b
        with ExitStack() as late:
            xq_pool = late.enter_context(tc.tile_pool(name="xq", bufs=2 * c.MQ + 2))
            ps2b = late.enter_context(tc.tile_pool(name="ps2b", bufs=2, space="PSUM"))
            av_pool = late.enter_context(tc.tile_pool(name="avt", bufs=2))
            acc_pool = late.enter_context(tc.tile_pool(name="acc", bufs=3))
            e_pool = late.enter_context(tc.tile_pool(name="e", bufs=4))
            rs_pool = late.enter_context(tc.tile_pool(name="rs", bufs=2))
            ev4 = late.enter_context(tc.tile_pool(name="ev4", bufs=3))
            s_ps = late.enter_context(tc.tile_pool(name="sps", bufs=2, space="PSUM"))
            av_ps = late.enter_context(tc.tile_pool(name="avps", bufs=1, space="PSUM"))
            sm_ps = late.enter_context(tc.tile_pool(name="smps", bufs=1, space="PSUM"))
            ps4 = late.enter_context(tc.tile_pool(name="ps4", bufs=2, space="PSUM"))

            def load_xq(n):
                xq = []
                for k in range(c.MQ):
                    t8 = xq_pool.tile([128, c.NC], FP8, tag="xq8", name=f"xq8{k}")
                    nc.sync.dma_start(
                        t8[:], g2_out[n * c.QL + k * 128:n * c.QL + (k + 1) * 128, :])
                    t = xq_pool.tile([128, c.NC], BF16, tag="xq", name=f"xq{k}")
                    nc.vector.tensor_copy(t[:], t8[:])  # fp8 -> bf16 off PE
                    xq.append(t)
                return xq

            xq_next = load_xq(0)
            avt = None
            for qn in range(c.NT):
                qsl = slice(qn * c.NC, (qn + 1) * c.NC)
                nkj = ((qn + 1) * c.NC) // 128
                # q_b for this quarter
                xq = xq_next
                for m in range(c.NQM):
                    ps = ps2b.tile([128, c.NC], F32, tag="ps")
                    for k in range(c.MQ):
                        nc.tensor.matmul(ps[:], qbw_sb[k][:, m * 128:(m + 1) * 128],
                                         xq[k][:],
                                         start=(k == 0), stop=(k == c.MQ - 1))
                    nc.scalar.copy(q_sb[m][:, qsl], ps[:])
                    if m >= c.NHC:
                        rope_cols(q_sb[m][:, qsl], qsl, ps2b)
                if qn + 1 < c.NT:
                    xq_next = load_xq(qn + 1)   # prefetch under attention
                # attention for this quarter
                if qn % 2 == 0:   # o_proj runs per qn-pair on [128,2NC] tiles
                    avt = [av_pool.tile([128, 2 * c.NC], BF16, tag=f"av{h}",
                                        name=f"av{h}") for h in range(c.NHC)]
                half = slice((qn % 2) * c.NC, (qn % 2 + 1) * c.NC)
                for h in range(c.NHC):
                    q_nope = q_sb[h]
                    qr_t = q_sb[c.NHC + (h * 64) // 128]
                    qr_r0 = (h * 64) % 128
                    pav = av_ps.tile([128, c.NC], F32, tag="pav")
                    acc = acc_pool.tile([128, c.NC], F32, tag="acc")
                    for kj in range(nkj):
                        ks = slice(kj * 128, (kj + 1) * 128)
                        pss = s_ps.tile([128, c.NC], F32, tag="pss")
                        nc.tensor.matmul(pss[:], knope[h][:, ks],
                                         q_nope[:, qsl], start=True, stop=False)
                        nc.tensor.matmul(pss[:], krope[qr_r0:qr_r0 + 64, ks],
                                         qr_t[qr_r0:qr_r0 + 64, qsl],
                                         start=False, stop=True)
                        e = e_pool.tile([128, c.NC], BF16, tag="e")
                        off = kj * 128 - qn * c.NC
                        if off >= 0:  # diagonal tile: causal mask
                            msl = mask_sb[:, c.NC - 128 - off:2 * c.NC - 128 - off]
                            nc.vector.tensor_add(e[:], pss[:], msl)
                            nc.scalar.activation(e[:], e[:], AF.Exp)
                        else:
                            nc.scalar.activation(e[:], pss[:], AF.Exp)
                        first, last = (kj == 0), (kj == nkj - 1)
                        nc.tensor.matmul(pav[:], vsb[kj][:, h * c.V:(h + 1) * c.V],
                                         e[:], start=first, stop=last)
                        if first:
                            nc.vector.tensor_copy(acc[:], e[:])
                        else:
                            nc.vector.tensor_add(acc[:], acc[:], e[:])
                    accb = e_pool.tile([128, c.NC], BF16, tag="accb")
                    nc.scalar.copy(accb[:], acc[:])
                    psm = sm_ps.tile([128, c.NC], F32, tag="psm")
                    nc.tensor.matmul(psm[:], ones[:], accb[:],
                                     start=True, stop=True)
                    rs = rs_pool.tile([128, c.NC], F32, tag="rs")
                    nc.vector.reciprocal_approx_fast(rs[:], psm[:])
                    nc.vector.tensor_mul(avt[h][:, half], pav[:], rs[:])
                if qn % 2 == 0:
                    continue
                # o_proj for this query-tile pair (2KB rows per outT write)
                psl = slice((qn - 1) * c.NC, (qn + 1) * c.NC)
                for m in range(c.KD):
                    ms = slice(m * 128, (m + 1) * 128)
                    ev = ev4.tile([128, 2 * c.NC], BF16, tag="ev")
                    for hf in range(2):     # two psum bufs pipeline evictions
                        fs = slice(hf * c.NC, (hf + 1) * c.NC)
                        ps = ps4.tile([128, c.NC], F32, tag="ps")
                        for h in range(c.NHC):
                            nc.tensor.matmul(ps[:], ow_sb[h][:, ms],
                                             avt[h][:, fs],
                                             start=(h == 0), stop=(h == c.NHC - 1))
                        nc.vector.tensor_copy(ev[:, fs], ps[:])
                    nc.sync.dma_start(outT[ms, psl], ev[:])

    nc.compile()
    return nc


# ---------------- host-side prep ----------------
def make_tables(c: Cfg):
    j = np.arange(32, dtype=np.float64)
    invf = c.rope_base ** (-j / 32.0)
    pos = np.arange(c.T, dtype=np.float64)
    f = pos[:, None] * invf[None, :]
    cosT = np.cos(f).T.astype(np.float32)
    sinT = np.sin(f).T.astype(np.float32)
    cos64 = np.concatenate([cosT, cosT], 0)
    sin64 = np.concatenate([-sinT, sinT], 0)   # sign-folded rotate-half
    cos2 = np.concatenate([cos64, cos64], 0).astype(ml_dtypes.bfloat16)
    sin2 = np.concatenate([sin64, sin64], 0).astype(ml_dtypes.bfloat16)
    r = np.arange(128)[:, None]
    cc = np.arange(c.MASKW)[None, :]
    maskt = np.where(cc >= r + (c.NC - 128), 0.0, -1e30).astype(np.float32)
    pm = np.zeros((128, 128), dtype=np.float32)
    for m in range(128):
        blk, j = (m // 64) * 64, m % 64
        pm[blk + (j + 32) % 64, m] = 1.0   # rotate-half partition permutation
    return cos2, sin2, maskt, pm.astype(ml_dtypes.bfloat16)


def make_core_inputs(c: Cfg, x, q_a_w, q_a_norm_w, q_b_w, kv_a_w, kv_norm_w,
                     kv_b_w, o_w, batch, quarter, heads):
    cos2, sin2, maskt, pm = make_tables(c)
    scale = 1.0 / math.sqrt(c.QH)
    w1 = np.concatenate([q_a_w, kv_a_w], axis=1)          # [D, W1C]
    W1C = w1.shape[1]
    NCH = (W1C + 127) // 128
    w1p = np.zeros((c.D, NCH * 128), dtype=w1.dtype)
    w1p[:, :W1C] = w1
    # [p, chunk, k, col]: contiguous per-(p, chunk) 4KB descriptor loads
    w1r = np.ascontiguousarray(
        w1p.reshape(c.KD, 128, NCH, 128).transpose(1, 2, 0, 3))
    xT = np.ascontiguousarray(
        x[batch].T[:, quarter * c.TQ:(quarter + 1) * c.TQ])
    qb = q_b_w.reshape(c.QL, -1, c.QH)
    qbw = np.concatenate([qb[:, h, :c.NOPE] for h in heads] +
                         [qb[:, h, c.NOPE:] for h in heads], axis=1)
    qbw = np.ascontiguousarray(qbw * q_a_norm_w[:, None] * scale)
    kvb = kv_b_w.reshape(c.KVL, -1, c.NOPE + c.V)
    kbw = np.concatenate([kvb[:, h, :c.NOPE] for h in heads], axis=1)
    kbw = np.ascontiguousarray(kbw * kv_norm_w[:, None])
    vbw = np.concatenate([kvb[:, h, c.NOPE:] for h in heads], axis=1)
    vbw = np.ascontiguousarray(vbw * kv_norm_w[:, None])
    o3 = o_w.reshape(-1, c.V, c.D)
    ows = np.ascontiguousarray(np.concatenate([o3[h] for h in heads], axis=0))
    bf = ml_dtypes.bfloat16
    return {'xT': xT.astype(bf), 'w1': w1r.astype(bf), 'qbw': qbw.astype(bf),
            'kbw': kbw.astype(bf), 'vbw': vbw.astype(bf), 'ow': ows.astype(bf),
            'cos2': cos2, 'sin2': sin2, 'maskt': maskt, 'perm': pm}


def prepare_in_maps(x, q_a_w, q_a_norm_w, q_b_w, kv_a_w, kv_norm_w, kv_b_w, o_w):
    args = [np.asarray(a, dtype=np.float32) for a in
            (x, q_a_w, q_a_norm_w, q_b_w, kv_a_w, kv_norm_w, kv_b_w, o_w)]
    in_maps = []
    for core in range(NCORES):
        b, g = core // GROUPS, core % GROUPS
        heads = list(range(g * NHC, (g + 1) * NHC))
        in_maps.append(make_core_inputs(FULL, *args, b, g, heads))
    return in_maps


def combine(results):
    out = np.zeros((B, T, D), dtype=np.float32)
    for core in range(NCORES):
        out[core // GROUPS] += results[core]['outT'].astype(np.float32).T
    return out


_NC_CACHE = None


def kernel(x, q_a_w, q_a_norm_w, q_b_w, kv_a_w, kv_norm_w, kv_b_w, o_w):
    global _NC_CACHE
    in_maps = prepare_in_maps(x, q_a_w, q_a_norm_w, q_b_w, kv_a_w, kv_norm_w,
                              kv_b_w, o_w)
    if _NC_CACHE is None:
        _NC_CACHE = build_nc()
    res = run_bass_kernel_spmd(_NC_CACHE, in_maps, core_ids=list(range(NCORES)))
    return combine(res.results)


# revision 5
# speedup vs baseline: 1.1381x; 1.1381x over previous
"""MLA multi-head latent attention kernel for 8 TRN2 NeuronCores, v2.

Sharding: 8 cores = 2 batches (DP) x 4 head-groups of 4 heads (TP).
v2 removes the 4x-replicated shared LoRA down-projection: each core computes
x @ [q_a | kv_a | k_rope] only for ITS T-quarter (pre-normalized, RMS rsqrt
applied before staging), then two HBM AllGathers across the 4-core TP group
(kv scratch first so kv_b starts while the larger q gather is in flight)
reassemble the full-T scratch. Everything else (q_b, kv_b, attention for the
core's 4 heads, partial o_proj) is head-sharded as before; host sums the 4
per-group partial outputs per batch.

Other changes vs the 702us baseline: q_b outputs stay SBUF-resident (no DRAM
round-trip), softmax denominators accumulate on DVE + one ones-matmul
(instead of a per-k-tile PE matmul), reciprocal runs on the Act engine (DVE
RECIPROCAL is ~4us/tile), o_proj is interleaved per query-tile with
attention, and the partial outputs are written bf16.
"""
import math
import sys
from contextlib import ExitStack
from dataclasses import dataclass

sys.path.insert(0, '/opt/trn_rl_repo')
import numpy as np
import ml_dtypes
import concourse.bass as bass
import concourse.bacc as bacc
import concourse.mybir as mybir
from concourse import tile
from concourse.bass_utils import run_bass_kernel_spmd

F32 = mybir.dt.float32
BF16 = mybir.dt.bfloat16
FP8 = mybir.dt.float8e4
AF = mybir.ActivationFunctionType


@dataclass
class Cfg:
    T: int = 2048
    D: int = 2048
    QL: int = 1536
    KVL: int = 512
    NHC: int = 4          # heads per core
    NOPE: int = 128
    ROPE: int = 64
    V: int = 128
    eps: float = 1e-6
    rope_base: float = 10000.0

    @property
    def NC(self):
        return min(512, self.T)

    @property
    def KD(self):
        return self.D // 128

    @property
    def MQ(self):
        return self.QL // 128

    @property
    def MKV(self):
        return self.KVL // 128

    @property
    def NT(self):
        return self.T // self.NC

    @property
    def TK(self):
        return self.T // 128

    @property
    def TQ(self):         # local T-quarter (phase-1 shard)
        return self.T // 4

    @property
    def MASKW(self):
        return 2 * self.NC - 128

    @property
    def QH(self):
        return self.NOPE + self.ROPE

    @property
    def NQM(self):        # q_b output 128-chunks
        return (self.NHC * self.QH) // 128

    @property
    def KVROWS(self):     # rows in the kv gather: kv_c + krope
        return self.KVL + self.ROPE


# full-scale problem constants (per harness contract)
B, T, D = 2, 2048, 2048
QL, KVL = 1536, 512
NHEADS, NOPE, ROPE, V = 16, 128, 64, 128
QH = NOPE + ROPE
NCORES = 8
GROUPS = 4
NHC = NHEADS // GROUPS
FULL = Cfg()
REPLICA_GROUPS = [[0, 1, 2, 3], [4, 5, 6, 7]]


def build_nc(c: Cfg = FULL, num_devices: int = NCORES):
    nc = bacc.Bacc("TRN2", target_bir_lowering=False, debug=False,
                   num_devices=num_devices)
    W1C = c.QL + c.KVL + c.ROPE

    NCH = (W1C + 127) // 128           # w1 output chunks (last is 64 wide)
    xT = nc.dram_tensor("xT", [c.D, c.TQ], BF16, kind="ExternalInput").ap()
    # w1 pre-tiled on host: [p, chunk, k, col] so each chunk load is one
    # contiguous 4KB-per-partition descriptor instead of 256B strided rows
    w1 = nc.dram_tensor("w1", [128, NCH, c.KD, 128], BF16,
                        kind="ExternalInput").ap()
    qbw = nc.dram_tensor("qbw", [c.QL, c.NHC * c.QH], BF16, kind="ExternalInput").ap()
    kbw = nc.dram_tensor("kbw", [c.KVL, c.NHC * 128], BF16, kind="ExternalInput").ap()
    vbw = nc.dram_tensor("vbw", [c.KVL, c.NHC * c.V], BF16, kind="ExternalInput").ap()
    ow = nc.dram_tensor("ow", [c.NHC * c.V, c.D], BF16, kind="ExternalInput").ap()
    cos2 = nc.dram_tensor("cos2", [128, c.T], BF16, kind="ExternalInput").ap()
    sin2 = nc.dram_tensor("sin2", [128, c.T], BF16, kind="ExternalInput").ap()
    perm = nc.dram_tensor("perm", [128, 128], BF16, kind="ExternalInput").ap()
    maskt = nc.dram_tensor("maskt", [128, c.MASKW], F32, kind="ExternalInput").ap()
    outT = nc.dram_tensor("outT", [c.D, c.T], BF16, kind="ExternalOutput").ap()

    m1 = []
    off = 0
    while off < W1C:
        sz = min(128, W1C - off)
        m1.append((off, sz))
        off += sz
    NKCH = len(m1) - c.MQ              # kv chunk count (incl krope partial)
    m_order = list(range(c.MQ, len(m1))) + list(range(c.MQ))  # kv chunks first

    with tile.TileContext(nc) as tc, ExitStack() as top:
        dram = top.enter_context(tc.tile_pool(name="dram", bufs=1, space="DRAM"))
        g1_in = dram.tile([c.KVROWS, c.TQ], BF16)
        g2_in = dram.tile([c.QL, c.TQ], FP8)
        g1_out = dram.tile([4 * c.KVROWS, c.TQ], BF16)
        g2_out = dram.tile([4 * c.QL, c.TQ], FP8)

        const = top.enter_context(tc.tile_pool(name="const", bufs=1))
        ones_f = const.tile([128, 128], F32)
        nc.vector.memset(ones_f[:], 1.0)
        ones = const.tile([128, 128], BF16)
        nc.vector.tensor_copy(ones[:], ones_f[:])
        eps_sb = const.tile([128, 1], F32)
        nc.vector.memset(eps_sb[:], float(c.eps))

        # persistent SBUF: per-head K/V, q_b outputs, rope tables, weights
        kvc = top.enter_context(tc.tile_pool(name="kvc", bufs=1))
        knope = [kvc.tile([128, c.T], BF16, tag=f"kn{i}", name=f"kn{i}")
                 for i in range(c.NHC)]
        krope = kvc.tile([128, c.T], BF16, tag="krope")  # duplicated halves
        vsb = [kvc.tile([128, c.NHC * c.V], BF16, tag=f"v{i}", name=f"v{i}")
               for i in range(c.TK)]
        q_sb = [kvc.tile([128, c.T], BF16, tag=f"qs{m}", name=f"qs{m}")
                for m in range(c.NQM)]

        tb_pool = top.enter_context(tc.tile_pool(name="ropetb", bufs=1))
        cos_sb = tb_pool.tile([128, c.T], BF16, tag="cos")
        sin_sb = tb_pool.tile([128, c.T], BF16, tag="sin")
        perm_sb = tb_pool.tile([128, 128], BF16, tag="perm")

        wpool = top.enter_context(tc.tile_pool(name="wts", bufs=1))
        kbw_sb = [wpool.tile([128, c.NHC * 128], BF16, tag=f"kbw{k}",
                             name=f"kbw{k}") for k in range(c.MKV)]
        vbw_sb = [wpool.tile([128, c.NHC * c.V], BF16, tag=f"vbw{k}",
                             name=f"vbw{k}") for k in range(c.MKV)]
        qbw_sb = [wpool.tile([128, c.NHC * c.QH], BF16, tag=f"qbw{k}",
                             name=f"qbw{k}") for k in range(c.MQ)]
        ow_sb = [wpool.tile([128, c.D], BF16, tag=f"ow{h}", name=f"ow{h}")
                 for h in range(c.NHC)]
        mask_sb = wpool.tile([128, c.MASKW], F32, tag="mask")

        # weight/table loads, interleaved 1-2 per phase-1 chunk iteration on
        # the sync queues (need-order: tables/mask, kv_b, q_b, o_proj)
        wload = ([(cos_sb, cos2), (sin_sb, sin2), (perm_sb, perm),
                  (mask_sb, maskt)]
                 + [(kbw_sb[k], kbw[k * 128:(k + 1) * 128, :])
                    for k in range(c.MKV)]
                 + [(vbw_sb[k], vbw[k * 128:(k + 1) * 128, :])
                    for k in range(c.MKV)]
                 + [(qbw_sb[k], qbw[k * 128:(k + 1) * 128, :])
                    for k in range(c.MQ)]
                 + [(ow_sb[h], ow[h * c.V:(h + 1) * c.V, :])
                    for h in range(c.NHC)])

        # ---------------- phase 1: local-quarter x @ [q_a | kv_a | k_rope] --
        with ExitStack() as p1:
            xt_pool = p1.enter_context(tc.tile_pool(name="xt", bufs=1))
            w1_pool = p1.enter_context(tc.tile_pool(name="w1", bufs=3))
            ev_pool = p1.enter_context(tc.tile_pool(name="p1ev", bufs=1))
            sq_pool = p1.enter_context(tc.tile_pool(name="p1sq", bufs=3))
            nrm_pool = p1.enter_context(tc.tile_pool(name="p1nrm", bufs=3))
            rsq_pool = p1.enter_context(tc.tile_pool(name="p1rsq", bufs=1))
            ps_pool = p1.enter_context(tc.tile_pool(name="p1ps", bufs=2, space="PSUM"))
            ssq_ps = p1.enter_context(tc.tile_pool(name="ssqps", bufs=2, space="PSUM"))

            xt_sb = [xt_pool.tile([128, c.TQ], BF16, tag=f"xt{k}", name=f"xt{k}")
                     for k in range(c.KD)]
            wt0 = w1_pool.tile([128, c.KD, 128], BF16, tag="w1t")
            nc.sync.dma_start(wt0[:], w1[:, m_order[0]])
            for k in range(4):   # rest issued inside the first chunk iteration
                nc.sync.dma_start(xt_sb[k][:], xT[k * 128:(k + 1) * 128, :])

            ev_sb = [ev_pool.tile([128, c.TQ], BF16, tag=f"ev{i}", name=f"ev{i}")
                     for i in range(len(m1))]
            rsq_q = rsq_pool.tile([128, c.TQ], F32, tag="rsq_q")
            rsq_kv = rsq_pool.tile([128, c.TQ], F32, tag="rsq_kv")
            ssq_q = ssq_ps.tile([128, c.TQ], F32, tag="ssq_q", name="ssq_q")
            ssq_kv = ssq_ps.tile([128, c.TQ], F32, tag="ssq_kv", name="ssq_kv")

            def flush_group(is_q):
                # rsq from accumulated ssq, normalize chunk tiles, stage to
                # the gather input, then launch the group's AllGather
                dim = c.QL if is_q else c.KVL
                tgt = rsq_q if is_q else rsq_kv
                ssq = ssq_q if is_q else ssq_kv
                nc.scalar.activation(tgt[:], ssq[:], AF.Sqrt,
                                     bias=eps_sb[:], scale=1.0 / dim)
                nc.vector.reciprocal_approx_fast(tgt[:], tgt[:])
                idxs = range(c.MQ) if is_q else range(c.MQ, c.MQ + c.MKV)
                gdst = g2_in if is_q else g1_in
                for j, mi in enumerate(idxs):
                    # q scratch is gathered in fp8 (halves the collective's
                    # serial transfer time; ~1.6e-2 end-to-end error, within
                    # the 2e-2 budget); kv scratch stays bf16
                    nt = nrm_pool.tile([128, c.TQ], FP8 if is_q else BF16,
                                       tag="ntq" if is_q else "nt")
                    nc.vector.tensor_mul(nt[:], ev_sb[mi][:], tgt[:])
                    nc.sync.dma_start(gdst[j * 128:(j + 1) * 128, :], nt[:])
                if not is_q:
                    # krope rows staged raw (not RMS-normalized)
                    kr = ev_sb[c.MQ + c.MKV]
                    nc.sync.dma_start(g1_in[c.KVL:c.KVL + c.ROPE, :],
                                      kr[0:c.ROPE, :])
                gin, gout = (g2_in, g2_out) if is_q else (g1_in, g1_out)
                # flatten to 1-D so the CC engine sees one contiguous
                # buffer (large bursts) instead of 1KB rows
                nc.gpsimd.collective_compute(
                    "AllGather", mybir.AluOpType.bypass,
                    replica_groups=REPLICA_GROUPS,
                    ins=[gin[:].rearrange("a b -> (a b)")],
                    outs=[gout[:].rearrange("a b -> (a b)")])

            NW = len(wload)
            for oi, mi in enumerate(m_order):
                m0, msz = m1[mi]
                if oi == 0:
                    wt = wt0
                else:
                    wt = w1_pool.tile([128, c.KD, 128], BF16, tag="w1t")
                    nc.sync.dma_start(wt[:], w1[:, mi])
                if oi == 0:
                    for k in range(4, c.KD):
                        nc.sync.dma_start(xt_sb[k][:],
                                          xT[k * 128:(k + 1) * 128, :])
                lo = NW * oi // len(m_order)
                hi = NW * (oi + 1) // len(m_order)
                for dst, src in wload[lo:hi]:
                    nc.sync.dma_start(dst[:], src)
                ps = ps_pool.tile([128, c.TQ], F32, tag="ps")
                for k in range(c.KD):
                    nc.tensor.matmul(ps[:msz, :], wt[:, k, :msz],
                                     xt_sb[k][:],
                                     start=(k == 0), stop=(k == c.KD - 1))
                nc.scalar.copy(ev_sb[mi][:msz, :], ps[:msz, :])
                is_q = mi < c.MQ
                is_kr = m0 >= c.QL + c.KVL
                if not is_kr:
                    sq = sq_pool.tile([128, c.TQ], BF16, tag="sq")
                    nc.scalar.square(sq[:msz, :], ps[:msz, :])
                    ssq = ssq_q if is_q else ssq_kv
                    nmax = c.MQ if is_q else c.MKV
                    mloc = mi if is_q else mi - c.MQ
                    nc.tensor.matmul(ssq[:], ones[:msz, :], sq[:msz, :],
                                     start=(mloc == 0), stop=(mloc == nmax - 1))
                if oi == NKCH - 1:      # all kv chunks (incl krope) evicted
                    flush_group(is_q=False)
            flush_group(is_q=True)

        # ---------------- phases 2a/2b/3/4 ----------------
        rt_pool = top.enter_context(tc.tile_pool(name="p2rt", bufs=4))

        def rope_cols(x_ap, ns, pspool):
            # in-place rotate-half on [128, NC] slice: the partition swap is
            # a PE permutation matmul (no SBUF-SBUF DMA round-trips); sign
            # is folded into the sin table
            pr = pspool.tile([128, c.NC], F32, tag="ps")
            nc.tensor.matmul(pr[:], perm_sb[:], x_ap, start=True, stop=True)
            tmp = rt_pool.tile([128, c.NC], BF16, tag="rtmp", name="rtmp")
            t1 = rt_pool.tile([128, c.NC], BF16, tag="rt1", name="rt1")
            nc.vector.tensor_mul(tmp[:], pr[:], sin_sb[:, ns])
            nc.vector.tensor_mul(t1[:], x_ap, cos_sb[:, ns])
            nc.vector.tensor_add(x_ap, t1[:], tmp[:])

        # ---- 2a: kv_b (reads the kv gather only) ----
        with ExitStack() as p2a:
            kvr_pool = p2a.enter_context(tc.tile_pool(name="kvr", bufs=1))
            ps2 = p2a.enter_context(tc.tile_pool(name="ps2", bufs=2, space="PSUM"))
            kv_raw = [kvr_pool.tile([128, c.T], BF16, tag=f"kvr{k}",
                                    name=f"kvr{k}") for k in range(c.MKV)]
            for qtr in range(4):        # quarter-major so knope starts early
                for k in range(c.MKV):
                    nc.sync.dma_start(
                        kv_raw[k][:, qtr * c.TQ:(qtr + 1) * c.TQ],
                        g1_out[qtr * c.KVROWS + k * 128:
                               qtr * c.KVROWS + (k + 1) * 128, :])
            for h in range(c.NHC):
                for n in range(c.NT):
                    ns = slice(n * c.NC, (n + 1) * c.NC)
                    ps = ps2.tile([128, c.NC], F32, tag="ps")
                    for k in range(c.MKV):
                        nc.tensor.matmul(ps[:], kbw_sb[k][:, h * 128:(h + 1) * 128],
                                         kv_raw[k][:, ns],
                                         start=(k == 0), stop=(k == c.MKV - 1))
                    nc.scalar.copy(knope[h][:, ns], ps[:])
            for qtr in range(4):
                src = g1_out[qtr * c.KVROWS + c.KVL:
                             qtr * c.KVROWS + c.KVL + c.ROPE, :]
                nc.sync.dma_start(krope[0:64, qtr * c.TQ:(qtr + 1) * c.TQ], src)
                nc.sync.dma_start(krope[64:128, qtr * c.TQ:(qtr + 1) * c.TQ], src)
            for n in range(c.NT):
                ns = slice(n * c.NC, (n + 1) * c.NC)
                rope_cols(krope[:, ns], ns, ps2)
            for m in range(c.TK):
                ms = slice(m * 128, (m + 1) * 128)
                ps = ps2.tile([128, c.NHC * c.V], F32, tag="ps")
                for k in range(c.MKV):
                    nc.tensor.matmul(ps[:], kv_raw[k][:, ms], vbw_sb[k][:],
                                     start=(k == 0), stop=(k == c.MKV - 1))
                nc.scalar.copy(vsb[m][:], ps[:])

        # ---- interleaved q_b(quarter) -> attention(qn) -> o_proj(pair) ----
        # the q AllGather delivers peer quarters progressively at CC speed;
        # consuming quarter n right before attention qn hides the CC tail
        # behind attention compute instead of stalling after phase 2b
        with ExitStack() as late:
            xq_pool = late.enter_context(tc.tile_pool(name="xq", bufs=2 * c.MQ + 2))
            ps2b = late.enter_context(tc.tile_pool(name="ps2b", bufs=2, space="PSUM"))
            av_pool = late.enter_context(tc.tile_pool(name="avt", bufs=2))
            acc_pool = late.enter_context(tc.tile_pool(name="acc", bufs=3))
            e_pool = late.enter_context(tc.tile_pool(name="e", bufs=4))
            rs_pool = late.enter_context(tc.tile_pool(name="rs", bufs=2))
            ev4 = late.enter_context(tc.tile_pool(name="ev4", bufs=3))
            s_ps = late.enter_context(tc.tile_pool(name="sps", bufs=2, space="PSUM"))
            av_ps = late.enter_context(tc.tile_pool(name="avps", bufs=1, space="PSUM"))
            sm_ps = late.enter_context(tc.tile_pool(name="smps", bufs=1, space="PSUM"))
            ps4 = late.enter_context(tc.tile_pool(name="ps4", bufs=2, space="PSUM"))

            def load_xq(n):
                xq = []
                for k in range(c.MQ):
                    t8 = xq_pool.tile([128, c.NC], FP8, tag="xq8", name=f"xq8{k}")
                    nc.sync.dma_start(
                        t8[:], g2_out[n * c.QL + k * 128:n * c.QL + (k + 1) * 128, :])
                    t = xq_pool.tile([128, c.NC], BF16, tag="xq", name=f"xq{k}")
                    nc.vector.tensor_copy(t[:], t8[:])  # fp8 -> bf16 off PE
                    xq.append(t)
                return xq

            xq_next = load_xq(0)
            avt = None
            for qn in range(c.NT):
                qsl = slice(qn * c.NC, (qn + 1) * c.NC)
                nkj = ((qn + 1) * c.NC) // 128
                # q_b for this quarter
                xq = xq_next
                for m in range(c.NQM):
                    ps = ps2b.tile([128, c.NC], F32, tag="ps")
                    for k in range(c.MQ):
                        nc.tensor.matmul(ps[:], qbw_sb[k][:, m * 128:(m + 1) * 128],
                                         xq[k][:],
                                         start=(k == 0), stop=(k == c.MQ - 1))
                    nc.scalar.copy(q_sb[m][:, qsl], ps[:])
                    if m >= c.NHC:
                        rope_cols(q_sb[m][:, qsl], qsl, ps2b)
                if qn + 1 < c.NT:
                    xq_next = load_xq(qn + 1)   # prefetch under attention
                # attention for this quarter
                if qn % 2 == 0:   # o_proj runs per qn-pair on [128,2NC] tiles
                    avt = [av_pool.tile([128, 2 * c.NC], BF16, tag=f"av{h}",
                                        name=f"av{h}") for h in range(c.NHC)]
                half = slice((qn % 2) * c.NC, (qn % 2 + 1) * c.NC)
                for h in range(c.NHC):
                    q_nope = q_sb[h]
                    qr_t = q_sb[c.NHC + (h * 64) // 128]
                    qr_r0 = (h * 64) % 128
                    pav = av_ps.tile([128, c.NC], F32, tag="pav")
                    acc = acc_pool.tile([128, c.NC], F32, tag="acc")
                    for kj in range(nkj):
                        ks = slice(kj * 128, (kj + 1) * 128)
                        pss = s_ps.tile([128, c.NC], F32, tag="pss")
                        nc.tensor.matmul(pss[:], knope[h][:, ks],
                                         q_nope[:, qsl], start=True, stop=False)
                        nc.tensor.matmul(pss[:], krope[qr_r0:qr_r0 + 64, ks],
                                         qr_t[qr_r0:qr_r0 + 64, qsl],
                                         start=False, stop=True)
                        e = e_pool.tile([128, c.NC], BF16, tag="e")
                        off = kj * 128 - qn * c.NC
                        if off >= 0:  # diagonal tile: causal mask
                            msl = mask_sb[:, c.NC - 128 - off:2 * c.NC - 128 - off]
                            nc.vector.tensor_add(e[:], pss[:], msl)
                            nc.scalar.activation(e[:], e[:], AF.Exp)
                        else:
                            nc.scalar.activation(e[:], pss[:], AF.Exp)
                        first, last = (kj == 0), (kj == nkj - 1)
                        nc.tensor.matmul(pav[:], vsb[kj][:, h * c.V:(h + 1) * c.V],
                                         e[:], start=first, stop=last)
                        if first:
                            nc.vector.tensor_copy(acc[:], e[:])
                        else:
                            nc.vector.tensor_add(acc[:], acc[:], e[:])
                    accb = e_pool.tile([128, c.NC], BF16, tag="accb")
                    nc.vector.tensor_copy(accb[:], acc[:])
                    psm = sm_ps.tile([128, c.NC], F32, tag="psm")
                    nc.tensor.matmul(psm[:], ones[:], accb[:],
                                     start=True, stop=True)
                    rs = rs_pool.tile([128, c.NC], F32, tag="rs")
                    nc.vector.reciprocal_approx_fast(rs[:], psm[:])
                    nc.vector.tensor_mul(avt[h][:, half], pav[:], rs[:])
                if qn % 2 == 0:
                    continue
                # o_proj for this query-tile pair (2KB rows per outT write)
                psl = slice((qn - 1) * c.NC, (qn + 1) * c.NC)
                for m in range(c.KD):
                    ms = slice(m * 128, (m + 1) * 128)
                    ev = ev4.tile([128, 2 * c.NC], BF16, tag="ev")
                    for hf in range(2):     # two psum bufs pipeline evictions
                        fs = slice(hf * c.NC, (hf + 1) * c.NC)
                        ps = ps4.tile([128, c.NC], F32, tag="ps")
                        for h in range(c.NHC):
                            nc.tensor.matmul(ps[:], ow_sb[h][:, ms],
                                             avt[h][:, fs],
                                             start=(h == 0), stop=(h == c.NHC - 1))
                        nc.vector.tensor_copy(ev[:, fs], ps[:])
                    nc.sync.dma_start(outT[ms, psl], ev[:])

    nc.compile()
    return nc


# ---------------- host-side prep ----------------
def make_tables(c: Cfg):
    j = np.arange(32, dtype=np.float64)
    invf = c.rope_base ** (-j / 32.0)
    pos = np.arange(c.T, dtype=np.float64)
    f = pos[:, None] * invf[None, :]
    cosT = np.cos(f).T.astype(np.float32)
    sinT = np.sin(f).T.astype(np.float32)
    cos64 = np.concatenate([cosT, cosT], 0)
    sin64 = np.concatenate([-sinT, sinT], 0)   # sign-folded rotate-half
    cos2 = np.concatenate([cos64, cos64], 0).astype(ml_dtypes.bfloat16)
    sin2 = np.concatenate([sin64, sin64], 0).astype(ml_dtypes.bfloat16)
    r = np.arange(128)[:, None]
    cc = np.arange(c.MASKW)[None, :]
    maskt = np.where(cc >= r + (c.NC - 128), 0.0, -1e30).astype(np.float32)
    pm = np.zeros((128, 128), dtype=np.float32)
    for m in range(128):
        blk, j = (m // 64) * 64, m % 64
        pm[blk + (j + 32) % 64, m] = 1.0   # rotate-half partition permutation
    return cos2, sin2, maskt, pm.astype(ml_dtypes.bfloat16)


def make_core_inputs(c: Cfg, x, q_a_w, q_a_norm_w, q_b_w, kv_a_w, kv_norm_w,
                     kv_b_w, o_w, batch, quarter, heads):
    cos2, sin2, maskt, pm = make_tables(c)
    scale = 1.0 / math.sqrt(c.QH)
    w1 = np.concatenate([q_a_w, kv_a_w], axis=1)          # [D, W1C]
    W1C = w1.shape[1]
    NCH = (W1C + 127) // 128
    w1p = np.zeros((c.D, NCH * 128), dtype=w1.dtype)
    w1p[:, :W1C] = w1
    # [p, chunk, k, col]: contiguous per-(p, chunk) 4KB descriptor loads
    w1r = np.ascontiguousarray(
        w1p.reshape(c.KD, 128, NCH, 128).transpose(1, 2, 0, 3))
    xT = np.ascontiguousarray(
        x[batch].T[:, quarter * c.TQ:(quarter + 1) * c.TQ])
    qb = q_b_w.reshape(c.QL, -1, c.QH)
    qbw = np.concatenate([qb[:, h, :c.NOPE] for h in heads] +
                         [qb[:, h, c.NOPE:] for h in heads], axis=1)
    qbw = np.ascontiguousarray(qbw * q_a_norm_w[:, None] * scale)
    kvb = kv_b_w.reshape(c.KVL, -1, c.NOPE + c.V)
    kbw = np.concatenate([kvb[:, h, :c.NOPE] for h in heads], axis=1)
    kbw = np.ascontiguousarray(kbw * kv_norm_w[:, None])
    vbw = np.concatenate([kvb[:, h, c.NOPE:] for h in heads], axis=1)
    vbw = np.ascontiguousarray(vbw * kv_norm_w[:, None])
    o3 = o_w.reshape(-1, c.V, c.D)
    ows = np.ascontiguousarray(np.concatenate([o3[h] for h in heads], axis=0))
    bf = ml_dtypes.bfloat16
    return {'xT': xT.astype(bf), 'w1': w1r.astype(bf), 'qbw': qbw.astype(bf),
            'kbw': kbw.astype(bf), 'vbw': vbw.astype(bf), 'ow': ows.astype(bf),
            'cos2': cos2, 'sin2': sin2, 'maskt': maskt, 'perm': pm}


def prepare_in_maps(x, q_a_w, q_a_norm_w, q_b_w, kv_a_w, kv_norm_w, kv_b_w, o_w):
    args = [np.asarray(a, dtype=np.float32) for a in
            (x, q_a_w, q_a_norm_w, q_b_w, kv_a_w, kv_norm_w, kv_b_w, o_w)]
    in_maps = []
    for core in range(NCORES):
        b, g = core // GROUPS, core % GROUPS
        heads = list(range(g * NHC, (g + 1) * NHC))
        in_maps.append(make_core_inputs(FULL, *args, b, g, heads))
    return in_maps


def combine(results):
    out = np.zeros((B, T, D), dtype=np.float32)
    for core in range(NCORES):
        out[core // GROUPS] += results[core]['outT'].astype(np.float32).T
    return out


_NC_CACHE = None


def kernel(x, q_a_w, q_a_norm_w, q_b_w, kv_a_w, kv_norm_w, kv_b_w, o_w):
    global _NC_CACHE
    in_maps = prepare_in_maps(x, q_a_w, q_a_norm_w, q_b_w, kv_a_w, kv_norm_w,
                              kv_b_w, o_w)
    if _NC_CACHE is None:
        _NC_CACHE = build_nc()
    res = run_bass_kernel_spmd(_NC_CACHE, in_maps, core_ids=list(range(NCORES)))
    return combine(res.results)


# revision 10
# speedup vs baseline: 1.1482x; 1.0089x over previous
"""MLA multi-head latent attention kernel for 8 TRN2 NeuronCores, v2.

Sharding: 8 cores = 2 batches (DP) x 4 head-groups of 4 heads (TP).
v2 removes the 4x-replicated shared LoRA down-projection: each core computes
x @ [q_a | kv_a | k_rope] only for ITS T-quarter (pre-normalized, RMS rsqrt
applied before staging), then two HBM AllGathers across the 4-core TP group
(kv scratch first so kv_b starts while the larger q gather is in flight)
reassemble the full-T scratch. Everything else (q_b, kv_b, attention for the
core's 4 heads, partial o_proj) is head-sharded as before; host sums the 4
per-group partial outputs per batch.

Other changes vs the 702us baseline: q_b outputs stay SBUF-resident (no DRAM
round-trip), softmax denominators accumulate on DVE + one ones-matmul
(instead of a per-k-tile PE matmul), reciprocal runs on the Act engine (DVE
RECIPROCAL is ~4us/tile), o_proj is interleaved per query-tile with
attention, and the partial outputs are written bf16.
"""
import math
import sys
from contextlib import ExitStack
from dataclasses import dataclass

sys.path.insert(0, '/opt/trn_rl_repo')
import numpy as np
import ml_dtypes
import concourse.bass as bass
import concourse.bacc as bacc
import concourse.mybir as mybir
from concourse import tile
from concourse.bass_utils import run_bass_kernel_spmd

F32 = mybir.dt.float32
BF16 = mybir.dt.bfloat16
FP8 = mybir.dt.float8e4
AF = mybir.ActivationFunctionType


@dataclass
class Cfg:
    T: int = 2048
    D: int = 2048
    QL: int = 1536
    KVL: int = 512
    NHC: int = 4          # heads per core
    NOPE: int = 128
    ROPE: int = 64
    V: int = 128
    eps: float = 1e-6
    rope_base: float = 10000.0

    @property
    def NC(self):
        return min(512, self.T)

    @property
    def KD(self):
        return self.D // 128

    @property
    def MQ(self):
        return self.QL // 128

    @property
    def MKV(self):
        return self.KVL // 128

    @property
    def NT(self):
        return self.T // self.NC

    @property
    def TK(self):
        return self.T // 128

    @property
    def TQ(self):         # local T-quarter (phase-1 shard)
        return self.T // 4

    @property
    def MASKW(self):
        return 2 * self.NC - 128

    @property
    def QH(self):
        return self.NOPE + self.ROPE

    @property
    def NQM(self):        # q_b output 128-chunks
        return (self.NHC * self.QH) // 128

    @property
    def KVROWS(self):     # rows in the kv gather: kv_c + krope
        return self.KVL + self.ROPE


# full-scale problem constants (per harness contract)
B, T, D = 2, 2048, 2048
QL, KVL = 1536, 512
NHEADS, NOPE, ROPE, V = 16, 128, 64, 128
QH = NOPE + ROPE
NCORES = 8
GROUPS = 4
NHC = NHEADS // GROUPS
FULL = Cfg()
REPLICA_GROUPS = [[0, 1, 2, 3], [4, 5, 6, 7]]


def build_nc(c: Cfg = FULL, num_devices: int = NCORES):
    nc = bacc.Bacc("TRN2", target_bir_lowering=False, debug=False,
                   num_devices=num_devices)
    W1C = c.QL + c.KVL + c.ROPE

    NCH = (W1C + 127) // 128           # w1 output chunks (last is 64 wide)
    xT = nc.dram_tensor("xT", [c.D, c.TQ], BF16, kind="ExternalInput").ap()
    # w1 pre-tiled on host: [p, chunk, k, col] so each chunk load is one
    # contiguous 4KB-per-partition descriptor instead of 256B strided rows
    w1 = nc.dram_tensor("w1", [128, NCH, c.KD, 128], BF16,
                        kind="ExternalInput").ap()
    qbw = nc.dram_tensor("qbw", [c.QL, c.NHC * c.QH], BF16, kind="ExternalInput").ap()
    kbw = nc.dram_tensor("kbw", [c.KVL, c.NHC * 128], BF16, kind="ExternalInput").ap()
    vbw = nc.dram_tensor("vbw", [c.KVL, c.NHC * c.V], BF16, kind="ExternalInput").ap()
    ow = nc.dram_tensor("ow", [c.NHC * c.V, c.D], BF16, kind="ExternalInput").ap()
    cos2 = nc.dram_tensor("cos2", [128, c.T], BF16, kind="ExternalInput").ap()
    sin2 = nc.dram_tensor("sin2", [128, c.T], BF16, kind="ExternalInput").ap()
    perm = nc.dram_tensor("perm", [128, 128], BF16, kind="ExternalInput").ap()
    maskt = nc.dram_tensor("maskt", [128, c.MASKW], F32, kind="ExternalInput").ap()
    outT = nc.dram_tensor("outT", [c.D, c.T], BF16, kind="ExternalOutput").ap()

    m1 = []
    off = 0
    while off < W1C:
        sz = min(128, W1C - off)
        m1.append((off, sz))
        off += sz
    NKCH = len(m1) - c.MQ              # kv chunk count (incl krope partial)
    m_order = list(range(c.MQ, len(m1))) + list(range(c.MQ))  # kv chunks first

    with tile.TileContext(nc) as tc, ExitStack() as top:
        dram = top.enter_context(tc.tile_pool(name="dram", bufs=1, space="DRAM"))
        g1_in = dram.tile([c.KVROWS, c.TQ], BF16)
        g2_in = dram.tile([c.QL, c.TQ], FP8)
        g1_out = dram.tile([4 * c.KVROWS, c.TQ], BF16)
        g2_out = dram.tile([4 * c.QL, c.TQ], FP8)

        const = top.enter_context(tc.tile_pool(name="const", bufs=1))
        ones_f = const.tile([128, 128], F32)
        nc.vector.memset(ones_f[:], 1.0)
        ones = const.tile([128, 128], BF16)
        nc.vector.tensor_copy(ones[:], ones_f[:])
        eps_sb = const.tile([128, 1], F32)
        nc.vector.memset(eps_sb[:], float(c.eps))

        # persistent SBUF: per-head K/V, q_b outputs, rope tables, weights
        kvc = top.enter_context(tc.tile_pool(name="kvc", bufs=1))
        knope = [kvc.tile([128, c.T], BF16, tag=f"kn{i}", name=f"kn{i}")
                 for i in range(c.NHC)]
        krope = kvc.tile([128, c.T], BF16, tag="krope")  # duplicated halves
        vsb = [kvc.tile([128, c.NHC * c.V], BF16, tag=f"v{i}", name=f"v{i}")
               for i in range(c.TK)]
        q_sb = [kvc.tile([128, c.T], BF16, tag=f"qs{m}", name=f"qs{m}")
                for m in range(c.NQM)]

        tb_pool = top.enter_context(tc.tile_pool(name="ropetb", bufs=1))
        cos_sb = tb_pool.tile([128, c.T], BF16, tag="cos")
        sin_sb = tb_pool.tile([128, c.T], BF16, tag="sin")
        perm_sb = tb_pool.tile([128, 128], BF16, tag="perm")

        wpool = top.enter_context(tc.tile_pool(name="wts", bufs=1))
        kbw_sb = [wpool.tile([128, c.NHC * 128], BF16, tag=f"kbw{k}",
                             name=f"kbw{k}") for k in range(c.MKV)]
        vbw_sb = [wpool.tile([128, c.NHC * c.V], BF16, tag=f"vbw{k}",
                             name=f"vbw{k}") for k in range(c.MKV)]
        qbw_sb = [wpool.tile([128, c.NHC * c.QH], BF16, tag=f"qbw{k}",
                             name=f"qbw{k}") for k in range(c.MQ)]
        ow_sb = [wpool.tile([128, c.D], BF16, tag=f"ow{h}", name=f"ow{h}")
                 for h in range(c.NHC)]
        mask_sb = wpool.tile([128, c.MASKW], F32, tag="mask")

        # weight/table loads, interleaved 1-2 per phase-1 chunk iteration on
        # the sync queues (need-order: tables/mask, kv_b, q_b, o_proj)
        wload = ([(cos_sb, cos2), (sin_sb, sin2), (perm_sb, perm),
                  (mask_sb, maskt)]
                 + [(kbw_sb[k], kbw[k * 128:(k + 1) * 128, :])
                    for k in range(c.MKV)]
                 + [(vbw_sb[k], vbw[k * 128:(k + 1) * 128, :])
                    for k in range(c.MKV)]
                 + [(qbw_sb[k], qbw[k * 128:(k + 1) * 128, :])
                    for k in range(c.MQ)]
                 + [(ow_sb[h], ow[h * c.V:(h + 1) * c.V, :])
                    for h in range(c.NHC)])

        # ---------------- phase 1: local-quarter x @ [q_a | kv_a | k_rope] --
        with ExitStack() as p1:
            xt_pool = p1.enter_context(tc.tile_pool(name="xt", bufs=1))
            w1_pool = p1.enter_context(tc.tile_pool(name="w1", bufs=3))
            ev_pool = p1.enter_context(tc.tile_pool(name="p1ev", bufs=1))
            sq_pool = p1.enter_context(tc.tile_pool(name="p1sq", bufs=3))
            nrm_pool = p1.enter_context(tc.tile_pool(name="p1nrm", bufs=3))
            rsq_pool = p1.enter_context(tc.tile_pool(name="p1rsq", bufs=1))
            ps_pool = p1.enter_context(tc.tile_pool(name="p1ps", bufs=2, space="PSUM"))
            ssq_ps = p1.enter_context(tc.tile_pool(name="ssqps", bufs=2, space="PSUM"))

            xt_sb = [xt_pool.tile([128, c.TQ], BF16, tag=f"xt{k}", name=f"xt{k}")
                     for k in range(c.KD)]
            wt0 = w1_pool.tile([128, c.KD, 128], BF16, tag="w1t")
            nc.sync.dma_start(wt0[:], w1[:, m_order[0]])
            for k in range(4):   # rest issued inside the first chunk iteration
                nc.sync.dma_start(xt_sb[k][:], xT[k * 128:(k + 1) * 128, :])

            ev_sb = [ev_pool.tile([128, c.TQ], BF16, tag=f"ev{i}", name=f"ev{i}")
                     for i in range(len(m1))]
            rsq_q = rsq_pool.tile([128, c.TQ], F32, tag="rsq_q")
            rsq_kv = rsq_pool.tile([128, c.TQ], F32, tag="rsq_kv")
            ssq_q = ssq_ps.tile([128, c.TQ], F32, tag="ssq_q", name="ssq_q")
            ssq_kv = ssq_ps.tile([128, c.TQ], F32, tag="ssq_kv", name="ssq_kv")

            def flush_group(is_q):
                # rsq from accumulated ssq, normalize chunk tiles, stage to
                # the gather input, then launch the group's AllGather
                dim = c.QL if is_q else c.KVL
                tgt = rsq_q if is_q else rsq_kv
                ssq = ssq_q if is_q else ssq_kv
                nc.scalar.activation(tgt[:], ssq[:], AF.Sqrt,
                                     bias=eps_sb[:], scale=1.0 / dim)
                nc.vector.reciprocal_approx_fast(tgt[:], tgt[:])
                idxs = range(c.MQ) if is_q else range(c.MQ, c.MQ + c.MKV)
                gdst = g2_in if is_q else g1_in
                for j, mi in enumerate(idxs):
                    # q scratch is gathered in fp8 (halves the collective's
                    # serial transfer time; ~1.6e-2 end-to-end error, within
                    # the 2e-2 budget); kv scratch stays bf16
                    nt = nrm_pool.tile([128, c.TQ], FP8 if is_q else BF16,
                                       tag="ntq" if is_q else "nt")
                    nc.vector.tensor_mul(nt[:], ev_sb[mi][:], tgt[:])
                    nc.sync.dma_start(gdst[j * 128:(j + 1) * 128, :], nt[:])
                if not is_q:
                    # krope rows staged raw (not RMS-normalized)
                    kr = ev_sb[c.MQ + c.MKV]
                    nc.sync.dma_start(g1_in[c.KVL:c.KVL + c.ROPE, :],
                                      kr[0:c.ROPE, :])
                gin, gout = (g2_in, g2_out) if is_q else (g1_in, g1_out)
                # flatten to 1-D so the CC engine sees one contiguous
                # buffer (large bursts) instead of 1KB rows
                nc.gpsimd.collective_compute(
                    "AllGather", mybir.AluOpType.bypass,
                    replica_groups=REPLICA_GROUPS,
                    ins=[gin[:].rearrange("a b -> (a b)")],
                    outs=[gout[:].rearrange("a b -> (a b)")])

            NW = len(wload)
            for oi, mi in enumerate(m_order):
                m0, msz = m1[mi]
                if oi == 0:
                    wt = wt0
                else:
                    wt = w1_pool.tile([128, c.KD, 128], BF16, tag="w1t")
                    nc.sync.dma_start(wt[:], w1[:, mi])
                if oi == 0:
                    for k in range(4, c.KD):
                        nc.sync.dma_start(xt_sb[k][:],
                                          xT[k * 128:(k + 1) * 128, :])
                lo = NW * oi // len(m_order)
                hi = NW * (oi + 1) // len(m_order)
                for dst, src in wload[lo:hi]:
                    nc.sync.dma_start(dst[:], src)
                ps = ps_pool.tile([128, c.TQ], F32, tag="ps")
                for k in range(c.KD):
                    nc.tensor.matmul(ps[:msz, :], wt[:, k, :msz],
                                     xt_sb[k][:],
                                     start=(k == 0), stop=(k == c.KD - 1))
                nc.scalar.copy(ev_sb[mi][:msz, :], ps[:msz, :])
                is_q = mi < c.MQ
                is_kr = m0 >= c.QL + c.KVL
                if not is_kr:
                    sq = sq_pool.tile([128, c.TQ], BF16, tag="sq")
                    nc.scalar.square(sq[:msz, :], ps[:msz, :])
                    ssq = ssq_q if is_q else ssq_kv
                    nmax = c.MQ if is_q else c.MKV
                    mloc = mi if is_q else mi - c.MQ
                    nc.tensor.matmul(ssq[:], ones[:msz, :], sq[:msz, :],
                                     start=(mloc == 0), stop=(mloc == nmax - 1))
                if oi == NKCH - 1:      # all kv chunks (incl krope) evicted
                    flush_group(is_q=False)
            flush_group(is_q=True)

        # ---------------- phases 2a/2b/3/4 ----------------
        rt_pool = top.enter_context(tc.tile_pool(name="p2rt", bufs=4))

        def rope_cols(x_ap, ns, pspool):
            # in-place rotate-half on [128, NC] slice: the partition swap is
            # a PE permutation matmul (no SBUF-SBUF DMA round-trips); sign
            # is folded into the sin table
            pr = pspool.tile([128, c.NC], F32, tag="ps")
            nc.tensor.matmul(pr[:], perm_sb[:], x_ap, start=True, stop=True)
            tmp = rt_pool.tile([128, c.NC], BF16, tag="rtmp", name="rtmp")
            t1 = rt_pool.tile([128, c.NC], BF16, tag="rt1", name="rt1")
            nc.vector.tensor_mul(tmp[:], pr[:], sin_sb[:, ns])
            nc.vector.tensor_mul(t1[:], x_ap, cos_sb[:, ns])
            nc.vector.tensor_add(x_ap, t1[:], tmp[:])

        # ---- 2a: kv_b (reads the kv gather only) ----
        with ExitStack() as p2a:
            kvr_pool = p2a.enter_context(tc.tile_pool(name="kvr", bufs=1))
            ps2 = p2a.enter_context(tc.tile_pool(name="ps2", bufs=2, space="PSUM"))
            kv_raw = [kvr_pool.tile([128, c.T], BF16, tag=f"kvr{k}",
                                    name=f"kvr{k}") for k in range(c.MKV)]
            for qtr in range(4):        # quarter-major so knope starts early
                for k in range(c.MKV):
                    nc.sync.dma_start(
                        kv_raw[k][:, qtr * c.TQ:(qtr + 1) * c.TQ],
                        g1_out[qtr * c.KVROWS + k * 128:
                               qtr * c.KVROWS + (k + 1) * 128, :])
            for h in range(c.NHC):
                for n in range(c.NT):
                    ns = slice(n * c.NC, (n + 1) * c.NC)
                    ps = ps2.tile([128, c.NC], F32, tag="ps")
                    for k in range(c.MKV):
                        nc.tensor.matmul(ps[:], kbw_sb[k][:, h * 128:(h + 1) * 128],
                                         kv_raw[k][:, ns],
                                         start=(k == 0), stop=(k == c.MKV - 1))
                    nc.scalar.copy(knope[h][:, ns], ps[:])
            for qtr in range(4):
                src = g1_out[qtr * c.KVROWS + c.KVL:
                             qtr * c.KVROWS + c.KVL + c.ROPE, :]
                nc.sync.dma_start(krope[0:64, qtr * c.TQ:(qtr + 1) * c.TQ], src)
                nc.sync.dma_start(krope[64:128, qtr * c.TQ:(qtr + 1) * c.TQ], src)
            for n in range(c.NT):
                ns = slice(n * c.NC, (n + 1) * c.NC)
                rope_cols(krope[:, ns], ns, ps2)
            for m in range(c.TK):
                ms = slice(m * 128, (m + 1) * 128)
                ps = ps2.tile([128, c.NHC * c.V], F32, tag="ps")
                for k in range(c.MKV):
                    nc.tensor.matmul(ps[:], kv_raw[k][:, ms], vbw_sb[k][:],
                                     start=(k == 0), stop=(k == c.MKV - 1))
                nc.scalar.copy(vsb[m][:], ps[:])

        # ---- interleaved q_b(quarter) -> attention(qn) -> o_proj(pair) ----
        # the q AllGather delivers peer quarters progressively at CC speed;
        # consuming quarter n right before attention qn hides the CC tail
        # behind attention compute instead of stalling after phase 2b
        with ExitStack() as late:
            xq_pool = late.enter_context(tc.tile_pool(name="xq", bufs=2 * c.MQ + 2))
            ps2b = late.enter_context(tc.tile_pool(name="ps2b", bufs=2, space="PSUM"))
            av_pool = late.enter_context(tc.tile_pool(name="avt", bufs=2))
            acc_pool = late.enter_context(tc.tile_pool(name="acc", bufs=4))
            e_pool = late.enter_context(tc.tile_pool(name="e", bufs=5))
            rs_pool = late.enter_context(tc.tile_pool(name="rs", bufs=2))
            ev4 = late.enter_context(tc.tile_pool(name="ev4", bufs=3))
            s_ps = late.enter_context(tc.tile_pool(name="sps", bufs=2, space="PSUM"))
            av_ps = late.enter_context(tc.tile_pool(name="avps", bufs=1, space="PSUM"))
            sm_ps = late.enter_context(tc.tile_pool(name="smps", bufs=1, space="PSUM"))
            ps4 = late.enter_context(tc.tile_pool(name="ps4", bufs=2, space="PSUM"))

            def load_xq(n):
                xq = []
                for k in range(c.MQ):
                    t8 = xq_pool.tile([128, c.NC], FP8, tag="xq8", name=f"xq8{k}")
                    nc.sync.dma_start(
                        t8[:], g2_out[n * c.QL + k * 128:n * c.QL + (k + 1) * 128, :])
                    t = xq_pool.tile([128, c.NC], BF16, tag="xq", name=f"xq{k}")
                    nc.vector.tensor_copy(t[:], t8[:])  # fp8 -> bf16 off PE
                    xq.append(t)
                return xq

            xq_next = load_xq(0)
            avt = None
            for qn in range(c.NT):
                qsl = slice(qn * c.NC, (qn + 1) * c.NC)
                nkj = ((qn + 1) * c.NC) // 128
                # q_b for this quarter
                xq = xq_next
                for m in range(c.NQM):
                    ps = ps2b.tile([128, c.NC], F32, tag="ps")
                    for k in range(c.MQ):
                        nc.tensor.matmul(ps[:], qbw_sb[k][:, m * 128:(m + 1) * 128],
                                         xq[k][:],
                                         start=(k == 0), stop=(k == c.MQ - 1))
                    nc.scalar.copy(q_sb[m][:, qsl], ps[:])
                    if m >= c.NHC:
                        rope_cols(q_sb[m][:, qsl], qsl, ps2b)
                if qn + 1 < c.NT:
                    xq_next = load_xq(qn + 1)   # prefetch under attention
                # attention for this quarter
                if qn % 2 == 0:   # o_proj runs per qn-pair on [128,2NC] tiles
                    avt = [av_pool.tile([128, 2 * c.NC], BF16, tag=f"av{h}",
                                        name=f"av{h}") for h in range(c.NHC)]
                half = slice((qn % 2) * c.NC, (qn % 2 + 1) * c.NC)
                for h in range(c.NHC):
                    q_nope = q_sb[h]
                    qr_t = q_sb[c.NHC + (h * 64) // 128]
                    qr_r0 = (h * 64) % 128
                    pav = av_ps.tile([128, c.NC], F32, tag="pav")
                    acc = acc_pool.tile([128, c.NC], F32, tag="acc")
                    for kj in range(nkj):
                        ks = slice(kj * 128, (kj + 1) * 128)
                        pss = s_ps.tile([128, c.NC], F32, tag="pss")
                        nc.tensor.matmul(pss[:], knope[h][:, ks],
                                         q_nope[:, qsl], start=True, stop=False)
                        nc.tensor.matmul(pss[:], krope[qr_r0:qr_r0 + 64, ks],
                                         qr_t[qr_r0:qr_r0 + 64, qsl],
                                         start=False, stop=True)
                        e = e_pool.tile([128, c.NC], BF16, tag="e")
                        off = kj * 128 - qn * c.NC
                        if off >= 0:  # diagonal tile: causal mask
                            msl = mask_sb[:, c.NC - 128 - off:2 * c.NC - 128 - off]
                            nc.vector.tensor_add(e[:], pss[:], msl)
                            nc.scalar.activation(e[:], e[:], AF.Exp)
                        else:
                            nc.scalar.activation(e[:], pss[:], AF.Exp)
                        first, last = (kj == 0), (kj == nkj - 1)
                        nc.tensor.matmul(pav[:], vsb[kj][:, h * c.V:(h + 1) * c.V],
                                         e[:], start=first, stop=last)
                        if first:
                            nc.vector.tensor_copy(acc[:], e[:])
                        else:
                            nc.vector.tensor_add(acc[:], acc[:], e[:])
                    accb = e_pool.tile([128, c.NC], BF16, tag="accb")
                    nc.vector.tensor_copy(accb[:], acc[:])
                    psm = sm_ps.tile([128, c.NC], F32, tag="psm")
                    nc.tensor.matmul(psm[:], ones[:], accb[:],
                                     start=True, stop=True)
                    rs = rs_pool.tile([128, c.NC], F32, tag="rs")
                    nc.vector.reciprocal_approx_fast(rs[:], psm[:])
                    nc.vector.tensor_mul(avt[h][:, half], pav[:], rs[:])
                if qn % 2 == 0:
                    continue
                # o_proj for this query-tile pair (2KB rows per outT write)
                psl = slice((qn - 1) * c.NC, (qn + 1) * c.NC)
                for m in range(c.KD):
                    ms = slice(m * 128, (m + 1) * 128)
                    ev = ev4.tile([128, 2 * c.NC], BF16, tag="ev")
                    for hf in range(2):     # two psum bufs pipeline evictions
                        fs = slice(hf * c.NC, (hf + 1) * c.NC)
                        ps = ps4.tile([128, c.NC], F32, tag="ps")
                        for h in range(c.NHC):
                            nc.tensor.matmul(ps[:], ow_sb[h][:, ms],
                                             avt[h][:, fs],
                                             start=(h == 0), stop=(h == c.NHC - 1))
                        nc.vector.tensor_copy(ev[:, fs], ps[:])
                    nc.sync.dma_start(outT[ms, psl], ev[:])

    nc.compile()
    return nc


# ---------------- host-side prep ----------------
def make_tables(c: Cfg):
    j = np.arange(32, dtype=np.float64)
    invf = c.rope_base ** (-j / 32.0)
    pos = np.arange(c.T, dtype=np.float64)
    f = pos[:, None] * invf[None, :]
    cosT = np.cos(f).T.astype(np.float32)
    sinT = np.sin(f).T.astype(np.float32)
    cos64 = np.concatenate([cosT, cosT], 0)
    sin64 = np.concatenate([-sinT, sinT], 0)   # sign-folded rotate-half
    cos2 = np.concatenate([cos64, cos64], 0).astype(ml_dtypes.bfloat16)
    sin2 = np.concatenate([sin64, sin64], 0).astype(ml_dtypes.bfloat16)
    r = np.arange(128)[:, None]
    cc = np.arange(c.MASKW)[None, :]
    maskt = np.where(cc >= r + (c.NC - 128), 0.0, -1e30).astype(np.float32)
    pm = np.zeros((128, 128), dtype=np.float32)
    for m in range(128):
        blk, j = (m // 64) * 64, m % 64
        pm[blk + (j + 32) % 64, m] = 1.0   # rotate-half partition permutation
    return cos2, sin2, maskt, pm.astype(ml_dtypes.bfloat16)


def make_core_inputs(c: Cfg, x, q_a_w, q_a_norm_w, q_b_w, kv_a_w, kv_norm_w,
                     kv_b_w, o_w, batch, quarter, heads):
    cos2, sin2, maskt, pm = make_tables(c)
    scale = 1.0 / math.sqrt(c.QH)
    w1 = np.concatenate([q_a_w, kv_a_w], axis=1)          # [D, W1C]
    W1C = w1.shape[1]
    NCH = (W1C + 127) // 128
    w1p = np.zeros((c.D, NCH * 128), dtype=w1.dtype)
    w1p[:, :W1C] = w1
    # [p, chunk, k, col]: contiguous per-(p, chunk) 4KB descriptor loads
    w1r = np.ascontiguousarray(
        w1p.reshape(c.KD, 128, NCH, 128).transpose(1, 2, 0, 3))
    xT = np.ascontiguousarray(
        x[batch].T[:, quarter * c.TQ:(quarter + 1) * c.TQ])
    qb = q_b_w.reshape(c.QL, -1, c.QH)
    qbw = np.concatenate([qb[:, h, :c.NOPE] for h in heads] +
                         [qb[:, h, c.NOPE:] for h in heads], axis=1)
    qbw = np.ascontiguousarray(qbw * q_a_norm_w[:, None] * scale)
    kvb = kv_b_w.reshape(c.KVL, -1, c.NOPE + c.V)
    kbw = np.concatenate([kvb[:, h, :c.NOPE] for h in heads], axis=1)
    kbw = np.ascontiguousarray(kbw * kv_norm_w[:, None])
    vbw = np.concatenate([kvb[:, h, c.NOPE:] for h in heads], axis=1)
    vbw = np.ascontiguousarray(vbw * kv_norm_w[:, None])
    o3 = o_w.reshape(-1, c.V, c.D)
    ows = np.ascontiguousarray(np.concatenate([o3[h] for h in heads], axis=0))
    bf = ml_dtypes.bfloat16
    return {'xT': xT.astype(bf), 'w1': w1r.astype(bf), 'qbw': qbw.astype(bf),
            'kbw': kbw.astype(bf), 'vbw': vbw.astype(bf), 'ow': ows.astype(bf),
            'cos2': cos2, 'sin2': sin2, 'maskt': maskt, 'perm': pm}


def prepare_in_maps(x, q_a_w, q_a_norm_w, q_b_w, kv_a_w, kv_norm_w, kv_b_w, o_w):
    args = [np.asarray(a, dtype=np.float32) for a in
            (x, q_a_w, q_a_norm_w, q_b_w, kv_a_w, kv_norm_w, kv_b_w, o_w)]
    in_maps = []
    for core in range(NCORES):
        b, g = core // GROUPS, core % GROUPS
        heads = list(range(g * NHC, (g + 1) * NHC))
        in_maps.append(make_core_inputs(FULL, *args, b, g, heads))
    return in_maps


def combine(results):
    out = np.zeros((B, T, D), dtype=np.float32)
    for core in range(NCORES):
        out[core // GROUPS] += results[core]['outT'].astype(np.float32).T
    return out


_NC_CACHE = None


def kernel(x, q_a_w, q_a_norm_w, q_b_w, kv_a_w, kv_norm_w, kv_b_w, o_w):
    global _NC_CACHE
    in_maps = prepare_in_maps(x, q_a_w, q_a_norm_w, q_b_w, kv_a_w, kv_norm_w,
                              kv_b_w, o_w)
    if _NC_CACHE is None:
        _NC_CACHE = build_nc()
    res = run_bass_kernel_spmd(_NC_CACHE, in_maps, core_ids=list(range(NCORES)))
    return combine(res.results)


# revision 11
# speedup vs baseline: 1.1522x; 1.0035x over previous
"""MLA multi-head latent attention kernel for 8 TRN2 NeuronCores, v2.

Sharding: 8 cores = 2 batches (DP) x 4 head-groups of 4 heads (TP).
v2 removes the 4x-replicated shared LoRA down-projection: each core computes
x @ [q_a | kv_a | k_rope] only for ITS T-quarter (pre-normalized, RMS rsqrt
applied before staging), then two HBM AllGathers across the 4-core TP group
(kv scratch first so kv_b starts while the larger q gather is in flight)
reassemble the full-T scratch. Everything else (q_b, kv_b, attention for the
core's 4 heads, partial o_proj) is head-sharded as before; host sums the 4
per-group partial outputs per batch.

Other changes vs the 702us baseline: q_b outputs stay SBUF-resident (no DRAM
round-trip), softmax denominators accumulate on DVE + one ones-matmul
(instead of a per-k-tile PE matmul), reciprocal runs on the Act engine (DVE
RECIPROCAL is ~4us/tile), o_proj is interleaved per query-tile with
attention, and the partial outputs are written bf16.
"""
import math
import sys
from contextlib import ExitStack
from dataclasses import dataclass

sys.path.insert(0, '/opt/trn_rl_repo')
import numpy as np
import ml_dtypes
import concourse.bass as bass
import concourse.bacc as bacc
import concourse.mybir as mybir
from concourse import tile
from concourse.bass_utils import run_bass_kernel_spmd

F32 = mybir.dt.float32
BF16 = mybir.dt.bfloat16
FP8 = mybir.dt.float8e4
AF = mybir.ActivationFunctionType


@dataclass
class Cfg:
    T: int = 2048
    D: int = 2048
    QL: int = 1536
    KVL: int = 512
    NHC: int = 4          # heads per core
    NOPE: int = 128
    ROPE: int = 64
    V: int = 128
    eps: float = 1e-6
    rope_base: float = 10000.0

    @property
    def NC(self):
        return min(512, self.T)

    @property
    def KD(self):
        return self.D // 128

    @property
    def MQ(self):
        return self.QL // 128

    @property
    def MKV(self):
        return self.KVL // 128

    @property
    def NT(self):
        return self.T // self.NC

    @property
    def TK(self):
        return self.T // 128

    @property
    def TQ(self):         # local T-quarter (phase-1 shard)
        return self.T // 4

    @property
    def MASKW(self):
        return 2 * self.NC - 128

    @property
    def QH(self):
        return self.NOPE + self.ROPE

    @property
    def NQM(self):        # q_b output 128-chunks
        return (self.NHC * self.QH) // 128

    @property
    def KVROWS(self):     # rows in the kv gather: kv_c + krope
        return self.KVL + self.ROPE


# full-scale problem constants (per harness contract)
B, T, D = 2, 2048, 2048
QL, KVL = 1536, 512
NHEADS, NOPE, ROPE, V = 16, 128, 64, 128
QH = NOPE + ROPE
NCORES = 8
GROUPS = 4
NHC = NHEADS // GROUPS
FULL = Cfg()
REPLICA_GROUPS = [[0, 1, 2, 3], [4, 5, 6, 7]]


def build_nc(c: Cfg = FULL, num_devices: int = NCORES):
    nc = bacc.Bacc("TRN2", target_bir_lowering=False, debug=False,
                   num_devices=num_devices)
    W1C = c.QL + c.KVL + c.ROPE

    NCH = (W1C + 127) // 128           # w1 output chunks (last is 64 wide)
    xT = nc.dram_tensor("xT", [c.D, c.TQ], BF16, kind="ExternalInput").ap()
    # w1 pre-tiled on host: [p, chunk, k, col] so each chunk load is one
    # contiguous 4KB-per-partition descriptor instead of 256B strided rows
    w1 = nc.dram_tensor("w1", [128, NCH, c.KD, 128], BF16,
                        kind="ExternalInput").ap()
    qbw = nc.dram_tensor("qbw", [c.QL, c.NHC * c.QH], BF16, kind="ExternalInput").ap()
    kbw = nc.dram_tensor("kbw", [c.KVL, c.NHC * 128], BF16, kind="ExternalInput").ap()
    vbw = nc.dram_tensor("vbw", [c.KVL, c.NHC * c.V], BF16, kind="ExternalInput").ap()
    ow = nc.dram_tensor("ow", [c.NHC * c.V, c.D], BF16, kind="ExternalInput").ap()
    cos2 = nc.dram_tensor("cos2", [128, c.T], BF16, kind="ExternalInput").ap()
    sin2 = nc.dram_tensor("sin2", [128, c.T], BF16, kind="ExternalInput").ap()
    perm = nc.dram_tensor("perm", [128, 128], BF16, kind="ExternalInput").ap()
    maskt = nc.dram_tensor("maskt", [128, c.MASKW], F32, kind="ExternalInput").ap()
    outT = nc.dram_tensor("outT", [c.D, c.T], BF16, kind="ExternalOutput").ap()

    m1 = []
    off = 0
    while off < W1C:
        sz = min(128, W1C - off)
        m1.append((off, sz))
        off += sz
    NKCH = len(m1) - c.MQ              # kv chunk count (incl krope partial)
    m_order = list(range(c.MQ, len(m1))) + list(range(c.MQ))  # kv chunks first

    with tile.TileContext(nc) as tc, ExitStack() as top:
        dram = top.enter_context(tc.tile_pool(name="dram", bufs=1, space="DRAM"))
        g1_in = dram.tile([c.KVROWS, c.TQ], BF16)
        g2_in = dram.tile([c.QL, c.TQ], FP8)
        g1_out = dram.tile([4 * c.KVROWS, c.TQ], BF16)
        g2_out = dram.tile([4 * c.QL, c.TQ], FP8)

        const = top.enter_context(tc.tile_pool(name="const", bufs=1))
        ones_f = const.tile([128, 128], F32)
        nc.vector.memset(ones_f[:], 1.0)
        ones = const.tile([128, 128], BF16)
        nc.vector.tensor_copy(ones[:], ones_f[:])
        eps_sb = const.tile([128, 1], F32)
        nc.vector.memset(eps_sb[:], float(c.eps))

        # persistent SBUF: per-head K/V, q_b outputs, rope tables, weights
        kvc = top.enter_context(tc.tile_pool(name="kvc", bufs=1))
        knope = [kvc.tile([128, c.T], BF16, tag=f"kn{i}", name=f"kn{i}")
                 for i in range(c.NHC)]
        krope = kvc.tile([128, c.T], BF16, tag="krope")  # duplicated halves
        vsb = [kvc.tile([128, c.NHC * c.V], BF16, tag=f"v{i}", name=f"v{i}")
               for i in range(c.TK)]
        q_sb = [kvc.tile([128, c.T], BF16, tag=f"qs{m}", name=f"qs{m}")
                for m in range(c.NQM)]

        tb_pool = top.enter_context(tc.tile_pool(name="ropetb", bufs=1))
        cos_sb = tb_pool.tile([128, c.T], BF16, tag="cos")
        sin_sb = tb_pool.tile([128, c.T], BF16, tag="sin")
        perm_sb = tb_pool.tile([128, 128], BF16, tag="perm")

        wpool = top.enter_context(tc.tile_pool(name="wts", bufs=1))
        kbw_sb = [wpool.tile([128, c.NHC * 128], BF16, tag=f"kbw{k}",
                             name=f"kbw{k}") for k in range(c.MKV)]
        vbw_sb = [wpool.tile([128, c.NHC * c.V], BF16, tag=f"vbw{k}",
                             name=f"vbw{k}") for k in range(c.MKV)]
        qbw_sb = [wpool.tile([128, c.NHC * c.QH], BF16, tag=f"qbw{k}",
                             name=f"qbw{k}") for k in range(c.MQ)]
        ow_sb = [wpool.tile([128, c.D], BF16, tag=f"ow{h}", name=f"ow{h}")
                 for h in range(c.NHC)]
        mask_sb = wpool.tile([128, c.MASKW], F32, tag="mask")

        # weight/table loads, interleaved 1-2 per phase-1 chunk iteration on
        # the sync queues (need-order: tables/mask, kv_b, q_b, o_proj)
        wload = ([(cos_sb, cos2), (sin_sb, sin2), (perm_sb, perm),
                  (mask_sb, maskt)]
                 + [(kbw_sb[k], kbw[k * 128:(k + 1) * 128, :])
                    for k in range(c.MKV)]
                 + [(vbw_sb[k], vbw[k * 128:(k + 1) * 128, :])
                    for k in range(c.MKV)]
                 + [(qbw_sb[k], qbw[k * 128:(k + 1) * 128, :])
                    for k in range(c.MQ)]
                 + [(ow_sb[h], ow[h * c.V:(h + 1) * c.V, :])
                    for h in range(c.NHC)])

        # ---------------- phase 1: local-quarter x @ [q_a | kv_a | k_rope] --
        with ExitStack() as p1:
            xt_pool = p1.enter_context(tc.tile_pool(name="xt", bufs=1))
            w1_pool = p1.enter_context(tc.tile_pool(name="w1", bufs=3))
            ev_pool = p1.enter_context(tc.tile_pool(name="p1ev", bufs=1))
            sq_pool = p1.enter_context(tc.tile_pool(name="p1sq", bufs=3))
            nrm_pool = p1.enter_context(tc.tile_pool(name="p1nrm", bufs=3))
            rsq_pool = p1.enter_context(tc.tile_pool(name="p1rsq", bufs=1))
            ps_pool = p1.enter_context(tc.tile_pool(name="p1ps", bufs=2, space="PSUM"))
            ssq_ps = p1.enter_context(tc.tile_pool(name="ssqps", bufs=2, space="PSUM"))

            xt_sb = [xt_pool.tile([128, c.TQ], BF16, tag=f"xt{k}", name=f"xt{k}")
                     for k in range(c.KD)]
            wt0 = w1_pool.tile([128, c.KD, 128], BF16, tag="w1t")
            nc.sync.dma_start(wt0[:], w1[:, m_order[0]])
            for k in range(4):   # rest issued inside the first chunk iteration
                nc.sync.dma_start(xt_sb[k][:], xT[k * 128:(k + 1) * 128, :])

            ev_sb = [ev_pool.tile([128, c.TQ], BF16, tag=f"ev{i}", name=f"ev{i}")
                     for i in range(len(m1))]
            rsq_q = rsq_pool.tile([128, c.TQ], F32, tag="rsq_q")
            rsq_kv = rsq_pool.tile([128, c.TQ], F32, tag="rsq_kv")
            ssq_q = ssq_ps.tile([128, c.TQ], F32, tag="ssq_q", name="ssq_q")
            ssq_kv = ssq_ps.tile([128, c.TQ], F32, tag="ssq_kv", name="ssq_kv")

            def flush_group(is_q):
                # rsq from accumulated ssq, normalize chunk tiles, stage to
                # the gather input, then launch the group's AllGather
                dim = c.QL if is_q else c.KVL
                tgt = rsq_q if is_q else rsq_kv
                ssq = ssq_q if is_q else ssq_kv
                nc.scalar.activation(tgt[:], ssq[:], AF.Sqrt,
                                     bias=eps_sb[:], scale=1.0 / dim)
                nc.vector.reciprocal_approx_fast(tgt[:], tgt[:])
                idxs = range(c.MQ) if is_q else range(c.MQ, c.MQ + c.MKV)
                gdst = g2_in if is_q else g1_in
                for j, mi in enumerate(idxs):
                    # q scratch is gathered in fp8 (halves the collective's
                    # serial transfer time; ~1.6e-2 end-to-end error, within
                    # the 2e-2 budget); kv scratch stays bf16
                    nt = nrm_pool.tile([128, c.TQ], FP8 if is_q else BF16,
                                       tag="ntq" if is_q else "nt")
                    nc.vector.tensor_mul(nt[:], ev_sb[mi][:], tgt[:])
                    nc.sync.dma_start(gdst[j * 128:(j + 1) * 128, :], nt[:])
                if not is_q:
                    # krope rows staged raw (not RMS-normalized)
                    kr = ev_sb[c.MQ + c.MKV]
                    nc.sync.dma_start(g1_in[c.KVL:c.KVL + c.ROPE, :],
                                      kr[0:c.ROPE, :])
                gin, gout = (g2_in, g2_out) if is_q else (g1_in, g1_out)
                # flatten to 1-D so the CC engine sees one contiguous
                # buffer (large bursts) instead of 1KB rows
                nc.gpsimd.collective_compute(
                    "AllGather", mybir.AluOpType.bypass,
                    replica_groups=REPLICA_GROUPS,
                    ins=[gin[:].rearrange("a b -> (a b)")],
                    outs=[gout[:].rearrange("a b -> (a b)")])

            NW = len(wload)
            for oi, mi in enumerate(m_order):
                m0, msz = m1[mi]
                if oi == 0:
                    wt = wt0
                else:
                    wt = w1_pool.tile([128, c.KD, 128], BF16, tag="w1t")
                    nc.sync.dma_start(wt[:], w1[:, mi])
                if oi == 0:
                    for k in range(4, c.KD):
                        nc.sync.dma_start(xt_sb[k][:],
                                          xT[k * 128:(k + 1) * 128, :])
                lo = NW * oi // len(m_order)
                hi = NW * (oi + 1) // len(m_order)
                for dst, src in wload[lo:hi]:
                    nc.sync.dma_start(dst[:], src)
                ps = ps_pool.tile([128, c.TQ], F32, tag="ps")
                for k in range(c.KD):
                    nc.tensor.matmul(ps[:msz, :], wt[:, k, :msz],
                                     xt_sb[k][:],
                                     start=(k == 0), stop=(k == c.KD - 1))
                nc.scalar.copy(ev_sb[mi][:msz, :], ps[:msz, :])
                is_q = mi < c.MQ
                is_kr = m0 >= c.QL + c.KVL
                if not is_kr:
                    sq = sq_pool.tile([128, c.TQ], BF16, tag="sq")
                    nc.scalar.square(sq[:msz, :], ps[:msz, :])
                    ssq = ssq_q if is_q else ssq_kv
                    nmax = c.MQ if is_q else c.MKV
                    mloc = mi if is_q else mi - c.MQ
                    nc.tensor.matmul(ssq[:], ones[:msz, :], sq[:msz, :],
                                     start=(mloc == 0), stop=(mloc == nmax - 1))
                if oi == NKCH - 1:      # all kv chunks (incl krope) evicted
                    flush_group(is_q=False)
            flush_group(is_q=True)

        # ---------------- phases 2a/2b/3/4 ----------------
        rt_pool = top.enter_context(tc.tile_pool(name="p2rt", bufs=4))

        def rope_cols(x_ap, ns, pspool):
            # in-place rotate-half on [128, NC] slice: the partition swap is
            # a PE permutation matmul (no SBUF-SBUF DMA round-trips); sign
            # is folded into the sin table
            pr = pspool.tile([128, c.NC], F32, tag="ps")
            nc.tensor.matmul(pr[:], perm_sb[:], x_ap, start=True, stop=True)
            tmp = rt_pool.tile([128, c.NC], BF16, tag="rtmp", name="rtmp")
            t1 = rt_pool.tile([128, c.NC], BF16, tag="rt1", name="rt1")
            nc.vector.tensor_mul(tmp[:], pr[:], sin_sb[:, ns])
            nc.vector.tensor_mul(t1[:], x_ap, cos_sb[:, ns])
            nc.vector.tensor_add(x_ap, t1[:], tmp[:])

        # ---- 2a: kv_b (reads the kv gather only) ----
        with ExitStack() as p2a:
            kvr_pool = p2a.enter_context(tc.tile_pool(name="kvr", bufs=1))
            ps2 = p2a.enter_context(tc.tile_pool(name="ps2", bufs=2, space="PSUM"))
            kv_raw = [kvr_pool.tile([128, c.T], BF16, tag=f"kvr{k}",
                                    name=f"kvr{k}") for k in range(c.MKV)]
            for qtr in range(4):        # quarter-major so knope starts early
                for k in range(c.MKV):
                    nc.sync.dma_start(
                        kv_raw[k][:, qtr * c.TQ:(qtr + 1) * c.TQ],
                        g1_out[qtr * c.KVROWS + k * 128:
                               qtr * c.KVROWS + (k + 1) * 128, :])
            for h in range(c.NHC):
                for n in range(c.NT):
                    ns = slice(n * c.NC, (n + 1) * c.NC)
                    ps = ps2.tile([128, c.NC], F32, tag="ps")
                    for k in range(c.MKV):
                        nc.tensor.matmul(ps[:], kbw_sb[k][:, h * 128:(h + 1) * 128],
                                         kv_raw[k][:, ns],
                                         start=(k == 0), stop=(k == c.MKV - 1))
                    nc.scalar.copy(knope[h][:, ns], ps[:])
            for qtr in range(4):
                src = g1_out[qtr * c.KVROWS + c.KVL:
                             qtr * c.KVROWS + c.KVL + c.ROPE, :]
                nc.sync.dma_start(krope[0:64, qtr * c.TQ:(qtr + 1) * c.TQ], src)
                nc.sync.dma_start(krope[64:128, qtr * c.TQ:(qtr + 1) * c.TQ], src)
            for n in range(c.NT):
                ns = slice(n * c.NC, (n + 1) * c.NC)
                rope_cols(krope[:, ns], ns, ps2)
            for m in range(c.TK):
                ms = slice(m * 128, (m + 1) * 128)
                ps = ps2.tile([128, c.NHC * c.V], F32, tag="ps")
                for k in range(c.MKV):
                    nc.tensor.matmul(ps[:], kv_raw[k][:, ms], vbw_sb[k][:],
                                     start=(k == 0), stop=(k == c.MKV - 1))
                nc.scalar.copy(vsb[m][:], ps[:])

        # ---- interleaved q_b(quarter) -> attention(qn) -> o_proj(pair) ----
        # the q AllGather delivers peer quarters progressively at CC speed;
        # consuming quarter n right before attention qn hides the CC tail
        # behind attention compute instead of stalling after phase 2b
        with ExitStack() as late:
            xq_pool = late.enter_context(tc.tile_pool(name="xq", bufs=2 * c.MQ + 2))
            ps2b = late.enter_context(tc.tile_pool(name="ps2b", bufs=2, space="PSUM"))
            av_pool = late.enter_context(tc.tile_pool(name="avt", bufs=2))
            acc_pool = late.enter_context(tc.tile_pool(name="acc", bufs=4))
            e_pool = late.enter_context(tc.tile_pool(name="e", bufs=6))
            rs_pool = late.enter_context(tc.tile_pool(name="rs", bufs=2))
            ev4 = late.enter_context(tc.tile_pool(name="ev4", bufs=3))
            s_ps = late.enter_context(tc.tile_pool(name="sps", bufs=2, space="PSUM"))
            av_ps = late.enter_context(tc.tile_pool(name="avps", bufs=1, space="PSUM"))
            sm_ps = late.enter_context(tc.tile_pool(name="smps", bufs=1, space="PSUM"))
            ps4 = late.enter_context(tc.tile_pool(name="ps4", bufs=2, space="PSUM"))

            def load_xq(n):
                xq = []
                for k in range(c.MQ):
                    t8 = xq_pool.tile([128, c.NC], FP8, tag="xq8", name=f"xq8{k}")
                    nc.sync.dma_start(
                        t8[:], g2_out[n * c.QL + k * 128:n * c.QL + (k + 1) * 128, :])
                    t = xq_pool.tile([128, c.NC], BF16, tag="xq", name=f"xq{k}")
                    nc.vector.tensor_copy(t[:], t8[:])  # fp8 -> bf16 off PE
                    xq.append(t)
                return xq

            xq_next = load_xq(0)
            avt = None
            for qn in range(c.NT):
                qsl = slice(qn * c.NC, (qn + 1) * c.NC)
                nkj = ((qn + 1) * c.NC) // 128
                # q_b for this quarter
                xq = xq_next
                for m in range(c.NQM):
                    ps = ps2b.tile([128, c.NC], F32, tag="ps")
                    for k in range(c.MQ):
                        nc.tensor.matmul(ps[:], qbw_sb[k][:, m * 128:(m + 1) * 128],
                                         xq[k][:],
                                         start=(k == 0), stop=(k == c.MQ - 1))
                    nc.scalar.copy(q_sb[m][:, qsl], ps[:])
                    if m >= c.NHC:
                        rope_cols(q_sb[m][:, qsl], qsl, ps2b)
                if qn + 1 < c.NT:
                    xq_next = load_xq(qn + 1)   # prefetch under attention
                # attention for this quarter
                if qn % 2 == 0:   # o_proj runs per qn-pair on [128,2NC] tiles
                    avt = [av_pool.tile([128, 2 * c.NC], BF16, tag=f"av{h}",
                                        name=f"av{h}") for h in range(c.NHC)]
                half = slice((qn % 2) * c.NC, (qn % 2 + 1) * c.NC)
                for h in range(c.NHC):
                    q_nope = q_sb[h]
                    qr_t = q_sb[c.NHC + (h * 64) // 128]
                    qr_r0 = (h * 64) % 128
                    pav = av_ps.tile([128, c.NC], F32, tag="pav")
                    acc = acc_pool.tile([128, c.NC], F32, tag="acc")
                    for kj in range(nkj):
                        ks = slice(kj * 128, (kj + 1) * 128)
                        pss = s_ps.tile([128, c.NC], F32, tag="pss")
                        nc.tensor.matmul(pss[:], knope[h][:, ks],
                                         q_nope[:, qsl], start=True, stop=False)
                        nc.tensor.matmul(pss[:], krope[qr_r0:qr_r0 + 64, ks],
                                         qr_t[qr_r0:qr_r0 + 64, qsl],
                                         start=False, stop=True)
                        e = e_pool.tile([128, c.NC], BF16, tag="e")
                        off = kj * 128 - qn * c.NC
                        if off >= 0:  # diagonal tile: causal mask
                            msl = mask_sb[:, c.NC - 128 - off:2 * c.NC - 128 - off]
                            nc.vector.tensor_add(e[:], pss[:], msl)
                            nc.scalar.activation(e[:], e[:], AF.Exp)
                        else:
                            nc.scalar.activation(e[:], pss[:], AF.Exp)
                        first, last = (kj == 0), (kj == nkj - 1)
                        nc.tensor.matmul(pav[:], vsb[kj][:, h * c.V:(h + 1) * c.V],
                                         e[:], start=first, stop=last)
                        if first:
                            nc.vector.tensor_copy(acc[:], e[:])
                        else:
                            nc.vector.tensor_add(acc[:], acc[:], e[:])
                    accb = e_pool.tile([128, c.NC], BF16, tag="accb")
                    nc.vector.tensor_copy(accb[:], acc[:])
                    psm = sm_ps.tile([128, c.NC], F32, tag="psm")
                    nc.tensor.matmul(psm[:], ones[:], accb[:],
                                     start=True, stop=True)
                    rs = rs_pool.tile([128, c.NC], F32, tag="rs")
                    nc.vector.reciprocal_approx_fast(rs[:], psm[:])
                    nc.vector.tensor_mul(avt[h][:, half], pav[:], rs[:])
                if qn % 2 == 0:
                    continue
                # o_proj for this query-tile pair (2KB rows per outT write)
                psl = slice((qn - 1) * c.NC, (qn + 1) * c.NC)
                for m in range(c.KD):
                    ms = slice(m * 128, (m + 1) * 128)
                    ev = ev4.tile([128, 2 * c.NC], BF16, tag="ev")
                    for hf in range(2):     # two psum bufs pipeline evictions
                        fs = slice(hf * c.NC, (hf + 1) * c.NC)
                        ps = ps4.tile([128, c.NC], F32, tag="ps")
                        for h in range(c.NHC):
                            nc.tensor.matmul(ps[:], ow_sb[h][:, ms],
                                             avt[h][:, fs],
                                             start=(h == 0), stop=(h == c.NHC - 1))
                        nc.vector.tensor_copy(ev[:, fs], ps[:])
                    nc.sync.dma_start(outT[ms, psl], ev[:])

    nc.compile()
    return nc


# ---------------- host-side prep ----------------
def make_tables(c: Cfg):
    j = np.arange(32, dtype=np.float64)
    invf = c.rope_base ** (-j / 32.0)
    pos = np.arange(c.T, dtype=np.float64)
    f = pos[:, None] * invf[None, :]
    cosT = np.cos(f).T.astype(np.float32)
    sinT = np.sin(f).T.astype(np.float32)
    cos64 = np.concatenate([cosT, cosT], 0)
    sin64 = np.concatenate([-sinT, sinT], 0)   # sign-folded rotate-half
    cos2 = np.concatenate([cos64, cos64], 0).astype(ml_dtypes.bfloat16)
    sin2 = np.concatenate([sin64, sin64], 0).astype(ml_dtypes.bfloat16)
    r = np.arange(128)[:, None]
    cc = np.arange(c.MASKW)[None, :]
    maskt = np.where(cc >= r + (c.NC - 128), 0.0, -1e30).astype(np.float32)
    pm = np.zeros((128, 128), dtype=np.float32)
    for m in range(128):
        blk, j = (m // 64) * 64, m % 64
        pm[blk + (j + 32) % 64, m] = 1.0   # rotate-half partition permutation
    return cos2, sin2, maskt, pm.astype(ml_dtypes.bfloat16)


def make_core_inputs(c: Cfg, x, q_a_w, q_a_norm_w, q_b_w, kv_a_w, kv_norm_w,
                     kv_b_w, o_w, batch, quarter, heads):
    cos2, sin2, maskt, pm = make_tables(c)
    scale = 1.0 / math.sqrt(c.QH)
    w1 = np.concatenate([q_a_w, kv_a_w], axis=1)          # [D, W1C]
    W1C = w1.shape[1]
    NCH = (W1C + 127) // 128
    w1p = np.zeros((c.D, NCH * 128), dtype=w1.dtype)
    w1p[:, :W1C] = w1
    # [p, chunk, k, col]: contiguous per-(p, chunk) 4KB descriptor loads
    w1r = np.ascontiguousarray(
        w1p.reshape(c.KD, 128, NCH, 128).transpose(1, 2, 0, 3))
    xT = np.ascontiguousarray(
        x[batch].T[:, quarter * c.TQ:(quarter + 1) * c.TQ])
    qb = q_b_w.reshape(c.QL, -1, c.QH)
    qbw = np.concatenate([qb[:, h, :c.NOPE] for h in heads] +
                         [qb[:, h, c.NOPE:] for h in heads], axis=1)
    qbw = np.ascontiguousarray(qbw * q_a_norm_w[:, None] * scale)
    kvb = kv_b_w.reshape(c.KVL, -1, c.NOPE + c.V)
    kbw = np.concatenate([kvb[:, h, :c.NOPE] for h in heads], axis=1)
    kbw = np.ascontiguousarray(kbw * kv_norm_w[:, None])
    vbw = np.concatenate([kvb[:, h, c.NOPE:] for h in heads], axis=1)
    vbw = np.ascontiguousarray(vbw * kv_norm_w[:, None])
    o3 = o_w.reshape(-1, c.V, c.D)
    ows = np.ascontiguousarray(np.concatenate([o3[h] for h in heads], axis=0))
    bf = ml_dtypes.bfloat16
    return {'xT': xT.astype(bf), 'w1': w1r.astype(bf), 'qbw': qbw.astype(bf),
            'kbw': kbw.astype(bf), 'vbw': vbw.astype(bf), 'ow': ows.astype(bf),
            'cos2': cos2, 'sin2': sin2, 'maskt': maskt, 'perm': pm}


def prepare_in_maps(x, q_a_w, q_a_norm_w, q_b_w, kv_a_w, kv_norm_w, kv_b_w, o_w):
    args = [np.asarray(a, dtype=np.float32) for a in
            (x, q_a_w, q_a_norm_w, q_b_w, kv_a_w, kv_norm_w, kv_b_w, o_w)]
    in_maps = []
    for core in range(NCORES):
        b, g = core // GROUPS, core % GROUPS
        heads = list(range(g * NHC, (g + 1) * NHC))
        in_maps.append(make_core_inputs(FULL, *args, b, g, heads))
    return in_maps


def combine(results):
    out = np.zeros((B, T, D), dtype=np.float32)
    for core in range(NCORES):
        out[core // GROUPS] += results[core]['outT'].astype(np.float32).T
    return out


_NC_CACHE = None


def kernel(x, q_a_w, q_a_norm_w, q_b_w, kv_a_w, kv_norm_w, kv_b_w, o_w):
    global _NC_CACHE
    in_maps = prepare_in_maps(x, q_a_w, q_a_norm_w, q_b_w, kv_a_w, kv_norm_w,
                              kv_b_w, o_w)
    if _NC_CACHE is None:
        _NC_CACHE = build_nc()
    res = run_bass_kernel_spmd(_NC_CACHE, in_maps, core_ids=list(range(NCORES)))
    return combine(res.results)
